# revision 1
# baseline (speedup 1.0000x reference)
"""HSIC loss kernel for 8 TRN2 NeuronCores.

Math: loss = -tr(CKW·CKG)/(n-1)^2 with CKX = KX·H, H = I - 1/n.
Expanded:  T  = S1 - (2/n)·Σ_i sW_i·sG_i + SW·SG/n²,  loss = -T/(n-1)²
where S1 = Σ_ij KW∘KG, sX = row sums of KX (KX symmetric).

The 2n×2n kernel matrix is only needed through its two diagonal blocks:
the cross blocks only enter via the bandwidth, and Σd2 has the closed
form 2N·Σsq - 2·||Σ_i x_i||², so bandwidth is computed on host.

Sharding: row-blocks of KW and KG. Core c computes rows [c·512, (c+1)·512)
of both 4096×4096 kernel blocks, reducing them on the fly to per-row
partial sums (Στ via ACT accum, Σ(τ²+τ⁴+τ⁸+τ¹⁶) via a custom DVE op,
Σ kW·kG via custom TENSOR_TENSOR_REDUCE). Host combines 8×[128,32]
partials in f64. No collectives needed.

Per out-tile [128,512]: PSUM = Σ_k WT[k,i]·WT[k,j] (4 bf16 matmuls)
+ (ã_i + ã_j) via one K=4 matmul with rows [1,1,ã_hi,ã_lo]/[ã_hi,ã_lo,1,1]
(ã = -sq/2 split hi/lo in bf16 so the add is f32-accurate), then
τ = Exp(P/(8bw)) on ACT = exp(-d2/(16bw)), and k = τ+τ²+τ⁴+τ⁸+τ¹⁶
= Σ_a exp(-d2/(bw·2^a)) via one custom DVE pass.
"""
import numpy as np
import ml_dtypes
from contextlib import ExitStack
from operator import add as _op_add

import concourse.bass as bass
import concourse.tile as tile
from concourse import bacc, mybir
import concourse.dve_ops as dve_ops
from concourse.dve_spec import Spec, Src0, Zero
from concourse.dve_ops import DveOp, _ref_body_sum

N_ROWS = 4096     # n
D = 512           # feature dim
NCORES = 8
ROWS_PER_CORE = N_ROWS // NCORES     # 512
P = 128
NM = ROWS_PER_CORE // P              # 4 row blocks per core
NJ = N_ROWS // 512                   # 8 column chunks of 512
NCOL = NJ * NM                       # 32 accum columns
KERNEL_NUM = 5
BF16 = ml_dtypes.bfloat16
LAST_RESULT = None
LAST_SCALE = None


def _ref_pows(in0, in1, c0, c1, c2):
    t = in0.astype(np.float32)
    t2 = t * t
    t4 = t2 * t2
    t8 = t4 * t4
    t16 = t8 * t8
    return (t2 + t4 + t8 + t16).astype(np.float32)


def _register_powsum():
    name = "POWSUM_HI_ANT"
    for op in dve_ops.OPS:
        if op.name == name:
            return op
    t = Src0
    t2 = t * t
    t4 = t2 * t2
    t8 = t4 * t4
    t16 = t8 * t8
    body = (t2 + t4) + (t8 + t16)
    spec = Spec(body=body, accum=_op_add, accum_init=Zero,
                reference=_ref_body_sum(_ref_pows))
    op = DveOp(name, spec, subdim=False,
               uops_sha={'v3': '250d8b54fc692992', 'v4': '05962d123e30a773'})
    dve_ops.OPS.append(op)
    dve_ops._SUB_OPCODE_FOR_NAME[name] = (
        dve_ops._CUSTOM_DVE_ROW_BASE + len(dve_ops.OPS) - 1)
    dve_ops.CUSTOM_DVE_SPECS[name] = op.spec
    return op


ADD_DVE_OF_8 = 2


def _build(scale: float):
    POWSUM = _register_powsum()
    f32 = mybir.dt.float32
    bf16 = mybir.dt.bfloat16
    nc = bacc.Bacc("TRN2", target_bir_lowering=False, debug=False)

    wt_d = nc.dram_tensor("wt", [D, N_ROWS], bf16, kind="ExternalInput")
    gt_d = nc.dram_tensor("gt", [D, N_ROWS], bf16, kind="ExternalInput")
    lw_d = nc.dram_tensor("lw", [D, ROWS_PER_CORE], bf16, kind="ExternalInput")
    lg_d = nc.dram_tensor("lg", [D, ROWS_PER_CORE], bf16, kind="ExternalInput")
    awr_d = nc.dram_tensor("awr", [4, N_ROWS], bf16, kind="ExternalInput")
    agr_d = nc.dram_tensor("agr", [4, N_ROWS], bf16, kind="ExternalInput")
    awl_d = nc.dram_tensor("awl", [4, ROWS_PER_CORE], bf16, kind="ExternalInput")
    agl_d = nc.dram_tensor("agl", [4, ROWS_PER_CORE], bf16, kind="ExternalInput")
    outs = {q: nc.dram_tensor(q, [P, NCOL], f32, kind="ExternalOutput")
            for q in ("acc_tw", "acc_sw", "acc_tg", "acc_sg", "acc_s1")}

    with tile.TileContext(nc) as tc, ExitStack() as ctx:
        const = ctx.enter_context(tc.tile_pool(name="const", bufs=1))
        rhsp = ctx.enter_context(tc.tile_pool(name="rhs", bufs=3))
        psum = ctx.enter_context(tc.tile_pool(name="psum", bufs=6, space="PSUM"))
        taup = ctx.enter_context(tc.tile_pool(name="tau", bufs=4))
        kp = ctx.enter_context(tc.tile_pool(name="kk", bufs=6))
        accp = ctx.enter_context(tc.tile_pool(name="acc", bufs=1))

        # persistent: lhsT slabs (4 partition blocks each), aug tiles, accum
        lw_t = [const.tile([P, ROWS_PER_CORE], bf16, tag=f"lw{kb}", name=f"lw{kb}") for kb in range(4)]
        lg_t = [const.tile([P, ROWS_PER_CORE], bf16, tag=f"lg{kb}", name=f"lg{kb}") for kb in range(4)]
        for kb in range(4):
            nc.sync.dma_start(lw_t[kb][:], lw_d.ap()[kb * P:(kb + 1) * P, :])
            nc.sync.dma_start(lg_t[kb][:], lg_d.ap()[kb * P:(kb + 1) * P, :])
        awr_t = const.tile([4, N_ROWS], bf16, tag="awr", name="awr_t")
        agr_t = const.tile([4, N_ROWS], bf16, tag="agr", name="agr_t")
        awl_t = const.tile([4, ROWS_PER_CORE], bf16, tag="awl", name="awl_t")
        agl_t = const.tile([4, ROWS_PER_CORE], bf16, tag="agl", name="agl_t")
        nc.sync.dma_start(awr_t[:], awr_d.ap()[:])
        nc.sync.dma_start(agr_t[:], agr_d.ap()[:])
        nc.sync.dma_start(awl_t[:], awl_d.ap()[:])
        nc.sync.dma_start(agl_t[:], agl_d.ap()[:])
        acc = {q: accp.tile([P, NCOL], f32, tag=q, name=q + "_t") for q in outs}

        for jc in range(NJ):
            rw = [rhsp.tile([P, 512], bf16, tag=f"rw{kb}", name=f"rw{kb}") for kb in range(4)]
            rg = [rhsp.tile([P, 512], bf16, tag=f"rg{kb}", name=f"rg{kb}") for kb in range(4)]
            for kb in range(4):
                nc.sync.dma_start(rw[kb][:], wt_d.ap()[kb * P:(kb + 1) * P,
                                                       jc * 512:(jc + 1) * 512])
                nc.sync.dma_start(rg[kb][:], gt_d.ap()[kb * P:(kb + 1) * P,
                                                       jc * 512:(jc + 1) * 512])
            for m in range(NM):
                col = jc * NM + m
                ktiles = {}
                for X, lhs, rhs, augl, augr in (("w", lw_t, rw, awl_t, awr_t),
                                                ("g", lg_t, rg, agl_t, agr_t)):
                    ps = psum.tile([P, 512], f32, tag="ps", name="ps")
                    for kb in range(4):
                        nc.tensor.matmul(ps[:], lhs[kb][:, m * P:(m + 1) * P],
                                         rhs[kb][:], start=(kb == 0), stop=False)
                    nc.tensor.matmul(ps[:], augl[:, m * P:(m + 1) * P],
                                     augr[:, jc * 512:(jc + 1) * 512],
                                     start=False, stop=True)
                    tau = taup.tile([P, 512], f32, tag="tau", name="tau")
                    nc.scalar.activation(tau[:], ps[:],
                                         mybir.ActivationFunctionType.Exp,
                                         bias=0.0, scale=scale,
                                         accum_out=acc["acc_t" + X][:, col:col + 1])
                    s = kp.tile([P, 512], f32, tag="s", name="s")
                    nc.vector._custom_dve(POWSUM, out=s[:], in0=tau[:],
                                          accum_out=acc["acc_s" + X][:, col:col + 1])
                    k = kp.tile([P, 512], f32, tag="k" + X, name="k" + X)
                    _ctr = jc * NM * 2 + m * 2 + (0 if X == "w" else 1)
                    if (_ctr % 8) < ADD_DVE_OF_8:
                        nc.vector.tensor_add(k[:], tau[:], s[:])
                    else:
                        nc.gpsimd.tensor_add(k[:], tau[:], s[:])
                    ktiles[X] = k
                dummy = kp.tile([P, 512], f32, tag="dummy", name="dummy")
                nc.vector._custom_dve(dve_ops.TENSOR_TENSOR_REDUCE, out=dummy[:],
                                      in0=ktiles["w"][:], in1=ktiles["g"][:],
                                      s0=0.0, s1=1.0,
                                      accum_out=acc["acc_s1"][:, col:col + 1])
        for q, d in outs.items():
            nc.sync.dma_start(d.ap()[:], acc[q][:])
    nc.compile()
    return nc


def _powsum5(t):
    t2 = t * t
    t4 = t2 * t2
    t8 = t4 * t4
    return t + t2 + t4 + t8 + t8 * t8


def kernel(W, G, **_):
    from concourse.bass_utils import run_bass_kernel_spmd
    W = np.asarray(W, dtype=np.float32)
    G = np.asarray(G, dtype=np.float32)
    n = W.shape[0]
    N = 2 * n

    # host prep (f64)
    W64, G64 = W.astype(np.float64), G.astype(np.float64)
    sqW = (W64 * W64).sum(1)
    sqG = (G64 * G64).sum(1)
    colsum = W64.sum(0) + G64.sum(0)
    sum_d2 = 2.0 * N * (sqW.sum() + sqG.sum()) - 2.0 * (colsum * colsum).sum()
    bw = sum_d2 / (N * N - N) / (2.0 ** (KERNEL_NUM // 2))
    scale = float(np.float32(1.0 / (8.0 * bw)))

    WTb = np.ascontiguousarray(W.T).astype(BF16)
    GTb = np.ascontiguousarray(G.T).astype(BF16)
    ones_row = np.ones(n, np.float64)

    def aug(sq):
        a = -0.5 * sq
        hi = a.astype(BF16)
        lo = (a - hi.astype(np.float64)).astype(BF16)
        return hi, lo
    awhi, awlo = aug(sqW)
    aghi, aglo = aug(sqG)
    awr = np.stack([awhi, awlo, ones_row.astype(BF16), ones_row.astype(BF16)])
    agr = np.stack([aghi, aglo, ones_row.astype(BF16), ones_row.astype(BF16)])

    global LAST_SCALE
    LAST_SCALE = scale
    nc = _build(scale)
    in_maps = []
    for c in range(NCORES):
        r0, r1 = c * ROWS_PER_CORE, (c + 1) * ROWS_PER_CORE
        o = np.ones(ROWS_PER_CORE, BF16)
        in_maps.append({
            "wt": WTb, "gt": GTb,
            "lw": np.ascontiguousarray(WTb[:, r0:r1]),
            "lg": np.ascontiguousarray(GTb[:, r0:r1]),
            "awr": awr, "agr": agr,
            "awl": np.stack([o, o, awhi[r0:r1], awlo[r0:r1]]),
            "agl": np.stack([o, o, aghi[r0:r1], aglo[r0:r1]]),
        })
    import os
    # NTFF profiling hook (antenv.axon_hooks) is absent in this container;
    # run_bass_kernel_spmd would crash resolving it if BASS_TRACE leaks in.
    os.environ["BASS_NEVER_TRACE"] = "1"
    res = run_bass_kernel_spmd(nc, in_maps, core_ids=list(range(NCORES)))
    global LAST_RESULT
    LAST_RESULT = res

    # host combine (f64)
    S1 = 0.0
    sW = np.zeros(n)
    sG = np.zeros(n)
    for c, out in enumerate(res.results):
        S1 += out["acc_s1"].astype(np.float64).sum()
        tw = out["acc_tw"].astype(np.float64) + out["acc_sw"].astype(np.float64)
        tg = out["acc_tg"].astype(np.float64) + out["acc_sg"].astype(np.float64)
        for m in range(NM):
            rows = slice(c * ROWS_PER_CORE + m * P, c * ROWS_PER_CORE + (m + 1) * P)
            sW[rows] = tw[:, m::NM].sum(1)
            sG[rows] = tg[:, m::NM].sum(1)

    # replace the (numerically noisy under bf16) diagonal with its exact value
    def diag_dev(Tb, ahi, alo):
        g_ii = (Tb.astype(np.float64) ** 2).sum(0)
        a2 = 2.0 * (ahi.astype(np.float64) + alo.astype(np.float64))
        return _powsum5(np.exp((g_ii + a2) * scale))
    kWd = diag_dev(WTb, awhi, awlo)
    kGd = diag_dev(GTb, aghi, aglo)
    S1 += (25.0 - kWd * kGd).sum()
    sW += 5.0 - kWd
    sG += 5.0 - kGd

    T = S1 - (2.0 / n) * (sW * sG).sum() + sW.sum() * sG.sum() / (n * n)
    loss = -T / ((n - 1) ** 2)
    return np.float32(loss)



# revision 12
# speedup vs baseline: 1.4407x; 1.4407x over previous
"""HSIC loss kernel for 8 TRN2 NeuronCores.

Math: loss = -tr(CKW.CKG)/(n-1)^2 with CKX = KX.H, H = I - 1/n.
Expanded:  T = S1 - (2/n) sum_i sW_i sG_i + SW SG/n^2, loss = -T/(n-1)^2
where S1 = sum_ij KW.KG, sX = row sums of KX (KX symmetric).

Symmetry: only the region R = {(i,j): j >= 512*floor(i/512)} of each 4096^2
kernel block is computed (144 [128,512] tiles per matrix instead of 256).
For elements below R, the mirror (strictly-upper 512-blocks) supplies them:
S1 doubles those tiles' contributions, and row sums get the mirrored part
from COLUMN sums of the computed tiles (ones-vector matmuls into PSUM).

Sharding: [128,512]-tile-rows r=0..31; core c owns rows {2c,2c+1,30-2c,31-2c}
= 18 (r,chunk) pairs/core, a perfectly balanced split. The SPMD program is
IDENTICAL on every core: 18 uniform steps; all per-core variation is data
(lhsT/rhs/aug streams staged in compute order by the host).

Per step (one (r,jc) pair, W and G halves side by side in PSUM [128,1024]):
fp8(e4m3) DoubleRow matmuls: 1 aug matmul (K=12: 4-way fp8 splits of
a_j = -sq_j/2 for both halves + per-row a_i via ones-selector rows) then
2 DR matmuls per half (K=256 each) accumulate the dot products. ACT does one
pair-wide Exp -> tau (f16). DVE: custom POWSUM4 (s = t^2+t^4+t^8+t^16), then
per half a 4x-mode scalar_tensor_tensor k = tau + s with accum = rowsum(k),
then one 4x STT kW*kG with accum -> S1 partials. A ones[128,1] matmul per
group accumulates column sums into per-group PSUM rows (partition 32-aligned
slots, flushed by 3 ACT copies). Host combines everything in f64 and replaces
the (quantized) diagonal with its exact value.
"""
import numpy as np
import ml_dtypes
from contextlib import ExitStack

import concourse.bass as bass
import concourse.tile as tile
from concourse import bacc, mybir
import concourse.dve_ops as dve_ops
from concourse.dve_spec import Spec, Src0, lower, _has_src1
from concourse.dve_ops import DveOp
from concourse.dve_uop import DveOpSpec

N_ROWS = 4096
D = 512
NCORES = 8
P = 128
NSTEP = 18
NG = 9
F8 = ml_dtypes.float8_e4m3
LAST_RESULT = None
LAST_SCALE = None

f32 = mybir.dt.float32
f16 = mybir.dt.float16
f8e4 = mybir.dt.float8e4
DR = mybir.MatmulPerfMode.DoubleRow
ADD = mybir.AluOpType.add
MULT = mybir.AluOpType.mult


def _ref_powsum4(in0, in1, s0, s1, imm2):
    t = in0.astype(np.float32)
    t2 = t * t
    t4 = t2 * t2
    t8 = t4 * t4
    return ((t2 + t4) + (t8 + t8 * t8)).astype(np.float32)


def _register_powsum4():
    name = "POWSUM4_HSIC_ANT"
    for op in dve_ops.OPS:
        if op.name == name:
            return op
    t = Src0
    t2 = t * t
    t4 = t2 * t2
    t8 = t4 * t4
    spec = Spec(body=(t2 + t4) + (t8 + t8 * t8), reference=_ref_powsum4)
    shas = {}
    for ver in ("v3", "v4"):
        tmp = DveOpSpec(name=name, opcode=1, uops=lower(spec, ver=ver),
                        rd1_en=_has_src1(spec))
        shas[ver] = tmp.sha(ver)
    op = DveOp(name, spec, subdim=False, uops_sha=shas)
    dve_ops.OPS.append(op)
    dve_ops._SUB_OPCODE_FOR_NAME[name] = (
        dve_ops._CUSTOM_DVE_ROW_BASE + len(dve_ops.OPS) - 1)
    dve_ops.CUSTOM_DVE_SPECS[name] = op.spec
    return op


def _schedule(c):
    """18 (tile_row, chunk, strict_upper) steps for core c, chunk-major.
    Consecutive step pairs (groups) always share the chunk."""
    rows = [2 * c, 2 * c + 1, 30 - 2 * c, 31 - 2 * c]
    steps = []
    for jc in range(8):
        for r in rows:
            if jc >= r // 4:
                steps.append((r, jc, jc > r // 4))
    assert len(steps) == NSTEP
    for g in range(NG):
        assert steps[2 * g][1] == steps[2 * g + 1][1]
        assert steps[2 * g][2] == steps[2 * g + 1][2]
    return steps


def _build(scale: float):
    POWSUM4 = _register_powsum4()
    nc = bacc.Bacc("TRN2", target_bir_lowering=False, debug=False)

    rw_d = nc.dram_tensor("rw", [P, NG * 2048], f8e4, kind="ExternalInput")
    rg_d = nc.dram_tensor("rg", [P, NG * 2048], f8e4, kind="ExternalInput")
    lw_d = nc.dram_tensor("lw", [P, NSTEP * 512], f8e4, kind="ExternalInput")
    lg_d = nc.dram_tensor("lg", [P, NSTEP * 512], f8e4, kind="ExternalInput")
    ar_d = nc.dram_tensor("ar", [12, NG * 2048], f8e4, kind="ExternalInput")
    al_d = nc.dram_tensor("al", [12, NSTEP * 256], f8e4, kind="ExternalInput")
    acc_d = nc.dram_tensor("acc", [P, 3 * NSTEP], f32, kind="ExternalOutput")
    cs_d = nc.dram_tensor("cs", [12, 1024], f32, kind="ExternalOutput")

    with tile.TileContext(nc) as tc, ExitStack() as ctx:
        const = ctx.enter_context(tc.tile_pool(name="const", bufs=1))
        psum = ctx.enter_context(tc.tile_pool(name="psum", bufs=2, space="PSUM"))
        csp = ctx.enter_context(tc.tile_pool(name="csp", bufs=1, space="PSUM"))
        taup = ctx.enter_context(tc.tile_pool(name="taup", bufs=2))
        spp = ctx.enter_context(tc.tile_pool(name="spp", bufs=2))
        kpp = ctx.enter_context(tc.tile_pool(name="kpp", bufs=2))
        dmp = ctx.enter_context(tc.tile_pool(name="dmp", bufs=2))

        rw_t = const.tile([P, NG * 2048], f8e4, tag="rw", name="rw_t")
        rg_t = const.tile([P, NG * 2048], f8e4, tag="rg", name="rg_t")
        lw_t = const.tile([P, NSTEP * 512], f8e4, tag="lw", name="lw_t")
        lg_t = const.tile([P, NSTEP * 512], f8e4, tag="lg", name="lg_t")
        ar_t = const.tile([12, NG * 2048], f8e4, tag="ar", name="ar_t")
        al_t = const.tile([12, NSTEP * 256], f8e4, tag="al", name="al_t")
        ones_t = const.tile([P, 1], f16, tag="ones", name="ones_t")
        acc_t = const.tile([P, 3 * NSTEP], f32, tag="acc", name="acc_t")
        stage = [const.tile([65, 1024], f32, tag=f"st{i}", name=f"st{i}")
                 for i in range(3)]
        nc.vector.memset(ones_t[:], 1.0)
        for g in range(NG):
            nc.sync.dma_start(rw_t[:, g * 2048:(g + 1) * 2048],
                              rw_d.ap()[:, g * 2048:(g + 1) * 2048])
            nc.sync.dma_start(rg_t[:, g * 2048:(g + 1) * 2048],
                              rg_d.ap()[:, g * 2048:(g + 1) * 2048])
            nc.sync.dma_start(lw_t[:, g * 1024:(g + 1) * 1024],
                              lw_d.ap()[:, g * 1024:(g + 1) * 1024])
            nc.sync.dma_start(lg_t[:, g * 1024:(g + 1) * 1024],
                              lg_d.ap()[:, g * 1024:(g + 1) * 1024])
            nc.sync.dma_start(ar_t[:, g * 2048:(g + 1) * 2048],
                              ar_d.ap()[:, g * 2048:(g + 1) * 2048])
            nc.sync.dma_start(al_t[:, g * 512:(g + 1) * 512],
                              al_d.ap()[:, g * 512:(g + 1) * 512])

        cs_tiles = [csp.tile([65, 1024], f32, tag=f"cs{i}", name=f"cs{i}")
                    for i in range(2)]
        flush_idx = 0
        for s in range(NSTEP):
            g, u = s // 2, s % 2
            ps = psum.tile([P, 1024], f32, tag="pair", name="pair")
            al_ap = al_t[:, s * 256:(s + 1) * 256].rearrange(
                "p (two m) -> p two m", two=2)
            for h in range(2):
                ar_ap = ar_t[:, g * 2048 + h * 1024:g * 2048 + (h + 1) * 1024] \
                    .rearrange("p (two n) -> p two n", two=2)
                nc.tensor.matmul(ps[:, h * 512:(h + 1) * 512], al_ap, ar_ap,
                                 start=True, stop=False, perf_mode=DR)
            for h, (l_t, r_t) in enumerate(((lw_t, rw_t), (lg_t, rg_t))):
                for kc in range(2):
                    lap = l_t[:, s * 512 + kc * 256:s * 512 + (kc + 1) * 256] \
                        .rearrange("p (two m) -> p two m", two=2)
                    rap = r_t[:, g * 2048 + kc * 1024:g * 2048 + (kc + 1) * 1024] \
                        .rearrange("p (two n) -> p two n", two=2)
                    nc.tensor.matmul(ps[:, h * 512:(h + 1) * 512], lap, rap,
                                     start=False, stop=(kc == 1), perf_mode=DR)
            tau = taup.tile([P, 1024], f16, tag="tau", name="tau")
            nc.scalar.activation(tau[:], ps[:],
                                 mybir.ActivationFunctionType.Exp,
                                 bias=0.0, scale=scale)
            sp = spp.tile([P, 1024], f16, tag="sp", name="sp")
            nc.vector._custom_dve(POWSUM4, out=sp[:], in0=tau[:])
            kp = kpp.tile([P, 1024], f16, tag="kp", name="kp")
            for h in range(2):
                sl = slice(h * 512, (h + 1) * 512)
                nc.vector.scalar_tensor_tensor(
                    out=kp[:, sl], in0=tau[:, sl], scalar=0.0, in1=sp[:, sl],
                    op0=ADD, op1=ADD, accum_out=acc_t[:, 3 * s + h:3 * s + h + 1])
            dummy = dmp.tile([P, 512], f16, tag="dm", name="dm")
            nc.vector.scalar_tensor_tensor(
                out=dummy[:], in0=kp[:, 0:512], scalar=1.0, in1=kp[:, 512:1024],
                op0=MULT, op1=MULT, accum_out=acc_t[:, 3 * s + 2:3 * s + 3])
            # column sums: one PSUM row per group, 32-aligned slots
            cs_cur = cs_tiles[(g // 3) % 2]
            q = (g % 3) * 32
            for h in range(2):
                nc.tensor.matmul(cs_cur[q:q + 1, h * 512:(h + 1) * 512],
                                 ones_t[:], kp[:, h * 512:(h + 1) * 512],
                                 start=(u == 0), stop=(u == 1))
            if u == 1 and g in (2, 5, 8):
                nc.scalar.copy(stage[flush_idx][:], cs_cur[:])
                flush_idx += 1
        for i in range(3):
            nc.sync.dma_start(cs_d.ap()[3 * i:3 * i + 3, :],
                              stage[i][0:65:32, :])
        nc.sync.dma_start(acc_d.ap()[:], acc_t[:])
    nc.compile()
    return nc


def _split4(x):
    """4-term fp8 split of x (f64): sum of returned rows ~ x."""
    outs = []
    r = x.copy()
    for _ in range(4):
        h = r.astype(F8)
        outs.append(h)
        r = r - h.astype(np.float64)
    return outs


def _powsum5_f32(t):
    """Device-replica: k from f16 tau, mirroring ACT/DVE/STT rounding."""
    t = t.astype(np.float32)
    t2 = t * t
    t4 = t2 * t2
    t8 = t4 * t4
    s = ((t2 + t4) + (t8 + t8 * t8)).astype(np.float16)
    pre = t + s.astype(np.float32)          # STT accum sees this (unrounded)
    k16 = pre.astype(np.float16)            # stored k tile
    return pre.astype(np.float64), k16.astype(np.float64)


def kernel(W, G, **_):
    import os
    os.environ["BASS_NEVER_TRACE"] = "1"
    from concourse.bass_utils import run_bass_kernel_spmd
    W = np.asarray(W, dtype=np.float32)
    G = np.asarray(G, dtype=np.float32)
    n = W.shape[0]
    N = 2 * n

    # bandwidth from the full-precision inputs (closed form, f64)
    W64, G64 = W.astype(np.float64), G.astype(np.float64)
    sqW_t = (W64 * W64).sum(1)
    sqG_t = (G64 * G64).sum(1)
    colsum = W64.sum(0) + G64.sum(0)
    sum_d2 = 2.0 * N * (sqW_t.sum() + sqG_t.sum()) - 2.0 * (colsum * colsum).sum()
    bw = sum_d2 / (N * N - N) / 4.0
    scale = float(np.float32(1.0 / (8.0 * bw)))

    # fp8 quantization + aug splits (from quantized rows: keeps d2_q >= 0
    # and the diagonal exactly zero pre-rounding)
    W8 = W.astype(F8)
    G8 = G.astype(F8)
    W8f = W8.astype(np.float64)
    G8f = G8.astype(np.float64)
    aW = -0.5 * (W8f * W8f).sum(1)
    aG = -0.5 * (G8f * G8f).sum(1)
    # 4-term fp8 split of a/2 (e4m3 max is 240; |a| can exceed it), applied
    # through selector rows of 2.0 in the aug matmul.
    aW4 = _split4(aW / 2.0)
    aG4 = _split4(aG / 2.0)
    aWs = 2.0 * sum(a.astype(np.float64) for a in aW4)
    aGs = 2.0 * sum(a.astype(np.float64) for a in aG4)
    W8T = np.ascontiguousarray(W8.T)  # [feat, row]
    G8T = np.ascontiguousarray(G8.T)

    scheds = [_schedule(c) for c in range(NCORES)]
    in_maps = []
    for c in range(NCORES):
        st = scheds[c]
        rw = np.zeros((P, NG * 2048), F8)
        rg = np.zeros((P, NG * 2048), F8)
        lw = np.zeros((P, NSTEP * 512), F8)
        lg = np.zeros((P, NSTEP * 512), F8)
        ar = np.zeros((12, NG * 2048), F8)
        al = np.zeros((12, NSTEP * 256), F8)
        for g in range(NG):
            jc = st[2 * g][1]
            cols = slice(jc * 512, (jc + 1) * 512)
            for q in range(4):
                rw[:, g * 2048 + q * 512:g * 2048 + (q + 1) * 512] = \
                    W8T[q * P:(q + 1) * P, cols]
                rg[:, g * 2048 + q * 512:g * 2048 + (q + 1) * 512] = \
                    G8T[q * P:(q + 1) * P, cols]
                # half-major aug rhs: [W: plane0|plane1, G: plane0|plane1]
                ar[q, g * 2048 + 0:g * 2048 + 512] = aW4[q][cols]
                ar[4 + q, g * 2048 + 0:g * 2048 + 512] = 2.0
                ar[q, g * 2048 + 1024:g * 2048 + 1536] = aG4[q][cols]
                ar[8 + q, g * 2048 + 1024:g * 2048 + 1536] = 2.0
        for s, (r, jc, su) in enumerate(st):
            rsl = slice(r * P, (r + 1) * P)
            for kc in range(2):
                for i in range(2):
                    fsl = slice(kc * 256 + i * P, kc * 256 + (i + 1) * P)
                    dst = slice(s * 512 + kc * 256 + i * P,
                                s * 512 + kc * 256 + (i + 1) * P)
                    lw[:, dst] = W8T[fsl, rsl]
                    lg[:, dst] = G8T[fsl, rsl]
            for q in range(4):
                al[q, s * 256:s * 256 + P] = 2.0
                al[4 + q, s * 256:s * 256 + P] = aW4[q][rsl]
                al[8 + q, s * 256:s * 256 + P] = aG4[q][rsl]
        in_maps.append({"rw": rw, "rg": rg, "lw": lw, "lg": lg,
                        "ar": ar, "al": al})

    global LAST_SCALE
    LAST_SCALE = scale
    nc = _build(scale)
    res = run_bass_kernel_spmd(nc, in_maps, core_ids=list(range(NCORES)))
    global LAST_RESULT
    LAST_RESULT = res

    # host combine (f64)
    S1 = 0.0
    sW = np.zeros(n)
    sG = np.zeros(n)
    for c in range(NCORES):
        out = res.results[c]
        acc = out["acc"].astype(np.float64)
        cs = out["cs"].astype(np.float64)
        for s, (r, jc, su) in enumerate(scheds[c]):
            rsl = slice(r * P, (r + 1) * P)
            sW[rsl] += acc[:, 3 * s + 0]
            sG[rsl] += acc[:, 3 * s + 1]
            S1 += (2.0 if su else 1.0) * acc[:, 3 * s + 2].sum()
        for g in range(NG):
            jc, su = scheds[c][2 * g][1], scheds[c][2 * g][2]
            if su:
                csl = slice(jc * 512, (jc + 1) * 512)
                sW[csl] += cs[g, 0:512]
                sG[csl] += cs[g, 512:1024]

    # diagonal: replace device-computed quantized values with exact 5.0
    sc32 = np.float32(scale)
    for X8f, aXs, sX, which in ((W8f, aWs, sW, 0), (G8f, aGs, sG, 1)):
        g_ii = (X8f * X8f).sum(1)
        P_ii = (g_ii + 2.0 * aXs).astype(np.float32)
        tau = (np.exp(P_ii * sc32)).astype(np.float16)
        pre, k16 = _powsum5_f32(tau)
        sX += 5.0 - pre
        if which == 0:
            kWd = k16
        else:
            kGd = k16
    S1 += (25.0 - kWd * kGd).sum()

    T = S1 - (2.0 / n) * (sW * sG).sum() + sW.sum() * sG.sum() / (n * n)
    loss = -T / ((n - 1) ** 2)
    return np.float32(loss)


# revision 16
# speedup vs baseline: 1.4584x; 1.0123x over previous
"""HSIC loss kernel for 8 TRN2 NeuronCores.

Math: loss = -tr(CKW.CKG)/(n-1)^2 with CKX = KX.H, H = I - 1/n.
Expanded:  T = S1 - (2/n) sum_i sW_i sG_i + SW SG/n^2, loss = -T/(n-1)^2
where S1 = sum_ij KW.KG, sX = row sums of KX (KX symmetric).

Symmetry: only the region R = {(i,j): j >= 512*floor(i/512)} of each 4096^2
kernel block is computed (144 [128,512] tiles per matrix instead of 256).
For elements below R, the mirror (strictly-upper 512-blocks) supplies them:
S1 doubles those tiles' contributions, and row sums get the mirrored part
from COLUMN sums of the computed tiles (ones-vector matmuls into PSUM).

Sharding: [128,512]-tile-rows r=0..31; core c owns rows {2c,2c+1,30-2c,31-2c}
= 18 (r,chunk) pairs/core, a perfectly balanced split. The SPMD program is
IDENTICAL on every core: 18 uniform steps; all per-core variation is data
(lhsT/rhs/aug streams staged in compute order by the host).

Per step (one (r,jc) pair, W and G halves side by side in PSUM [128,1024]):
fp8(e4m3) DoubleRow matmuls: 1 aug matmul (K=12: 4-way fp8 splits of
a_j = -sq_j/2 for both halves + per-row a_i via ones-selector rows) then
2 DR matmuls per half (K=256 each) accumulate the dot products. ACT does one
pair-wide Exp -> tau (f16). DVE: custom POWSUM4 (s = t^2+t^4+t^8+t^16), then
per half a 4x-mode scalar_tensor_tensor k = tau + s with accum = rowsum(k),
then one 4x STT kW*kG with accum -> S1 partials. A ones[128,1] matmul per
group accumulates column sums into per-group PSUM rows (partition 32-aligned
slots, flushed by 3 ACT copies). Host combines everything in f64 and replaces
the (quantized) diagonal with its exact value.
"""
import numpy as np
import ml_dtypes
from contextlib import ExitStack

import concourse.bass as bass
import concourse.tile as tile
from concourse import bacc, mybir
import concourse.dve_ops as dve_ops
from concourse.dve_spec import Spec, Src0, lower, _has_src1
from concourse.dve_ops import DveOp
from concourse.dve_uop import DveOpSpec

N_ROWS = 4096
D = 512
NCORES = 8
P = 128
NSTEP = 18
NG = 9
F8 = ml_dtypes.float8_e4m3
LAST_RESULT = None
LAST_SCALE = None

f32 = mybir.dt.float32
f16 = mybir.dt.float16
f8e4 = mybir.dt.float8e4
DR = mybir.MatmulPerfMode.DoubleRow
ADD = mybir.AluOpType.add
MULT = mybir.AluOpType.mult


def _ref_powsum5(in0, in1, s0, s1, imm2):
    t = in0.astype(np.float32)
    t2 = t * t
    t4 = t2 * t2
    t8 = t4 * t4
    return (((t + t2) + (t4 + t8)) + t8 * t8).astype(np.float32)


def _register_powsum5():
    name = "POWSUM5_HSIC_ANT"
    for op in dve_ops.OPS:
        if op.name == name:
            return op
    t = Src0
    t2 = t * t
    t4 = t2 * t2
    t8 = t4 * t4
    spec = Spec(body=((t + t2) + (t4 + t8)) + t8 * t8, reference=_ref_powsum5)
    shas = {}
    for ver in ("v3", "v4"):
        tmp = DveOpSpec(name=name, opcode=1, uops=lower(spec, ver=ver),
                        rd1_en=_has_src1(spec))
        shas[ver] = tmp.sha(ver)
    op = DveOp(name, spec, subdim=False, uops_sha=shas)
    dve_ops.OPS.append(op)
    dve_ops._SUB_OPCODE_FOR_NAME[name] = (
        dve_ops._CUSTOM_DVE_ROW_BASE + len(dve_ops.OPS) - 1)
    dve_ops.CUSTOM_DVE_SPECS[name] = op.spec
    return op


def _schedule(c):
    """18 (tile_row, chunk, strict_upper) steps for core c, chunk-major.
    Consecutive step pairs (groups) always share the chunk."""
    rows = [2 * c, 2 * c + 1, 30 - 2 * c, 31 - 2 * c]
    steps = []
    for jc in range(8):
        for r in rows:
            if jc >= r // 4:
                steps.append((r, jc, jc > r // 4))
    assert len(steps) == NSTEP
    for g in range(NG):
        assert steps[2 * g][1] == steps[2 * g + 1][1]
        assert steps[2 * g][2] == steps[2 * g + 1][2]
    return steps


def _build(scale: float):
    POWSUM5 = _register_powsum5()
    nc = bacc.Bacc("TRN2", target_bir_lowering=False, debug=False)

    rw_d = nc.dram_tensor("rw", [P, NG * 2048], f8e4, kind="ExternalInput")
    rg_d = nc.dram_tensor("rg", [P, NG * 2048], f8e4, kind="ExternalInput")
    lw_d = nc.dram_tensor("lw", [P, NSTEP * 512], f8e4, kind="ExternalInput")
    lg_d = nc.dram_tensor("lg", [P, NSTEP * 512], f8e4, kind="ExternalInput")
    ar_d = nc.dram_tensor("ar", [12, NG * 2048], f8e4, kind="ExternalInput")
    al_d = nc.dram_tensor("al", [12, NSTEP * 256], f8e4, kind="ExternalInput")
    acc_d = nc.dram_tensor("acc", [P, 3 * NSTEP], f32, kind="ExternalOutput")
    cs_d = nc.dram_tensor("cs", [12, 1024], f32, kind="ExternalOutput")

    with tile.TileContext(nc) as tc, ExitStack() as ctx:
        const = ctx.enter_context(tc.tile_pool(name="const", bufs=1))
        psum = ctx.enter_context(tc.tile_pool(name="psum", bufs=2, space="PSUM"))
        csp = ctx.enter_context(tc.tile_pool(name="csp", bufs=1, space="PSUM"))
        taup = ctx.enter_context(tc.tile_pool(name="taup", bufs=2))
        kpp = ctx.enter_context(tc.tile_pool(name="kpp", bufs=2))
        dmp = ctx.enter_context(tc.tile_pool(name="dmp", bufs=2))

        rw_t = const.tile([P, NG * 2048], f8e4, tag="rw", name="rw_t")
        rg_t = const.tile([P, NG * 2048], f8e4, tag="rg", name="rg_t")
        lw_t = const.tile([P, NSTEP * 512], f8e4, tag="lw", name="lw_t")
        lg_t = const.tile([P, NSTEP * 512], f8e4, tag="lg", name="lg_t")
        ar_t = const.tile([12, NG * 2048], f8e4, tag="ar", name="ar_t")
        al_t = const.tile([12, NSTEP * 256], f8e4, tag="al", name="al_t")
        ones_t = const.tile([P, 1], f16, tag="ones", name="ones_t")
        acc_t = const.tile([P, 3 * NSTEP], f32, tag="acc", name="acc_t")
        stage = [const.tile([65, 1024], f32, tag=f"st{i}", name=f"st{i}")
                 for i in range(3)]
        nc.vector.memset(ones_t[:], 1.0)
        for g in range(NG):
            nc.sync.dma_start(rw_t[:, g * 2048:(g + 1) * 2048],
                              rw_d.ap()[:, g * 2048:(g + 1) * 2048])
            nc.sync.dma_start(rg_t[:, g * 2048:(g + 1) * 2048],
                              rg_d.ap()[:, g * 2048:(g + 1) * 2048])
            nc.sync.dma_start(lw_t[:, g * 1024:(g + 1) * 1024],
                              lw_d.ap()[:, g * 1024:(g + 1) * 1024])
            nc.sync.dma_start(lg_t[:, g * 1024:(g + 1) * 1024],
                              lg_d.ap()[:, g * 1024:(g + 1) * 1024])
            nc.sync.dma_start(ar_t[:, g * 2048:(g + 1) * 2048],
                              ar_d.ap()[:, g * 2048:(g + 1) * 2048])
            nc.sync.dma_start(al_t[:, g * 512:(g + 1) * 512],
                              al_d.ap()[:, g * 512:(g + 1) * 512])

        cs_tiles = [csp.tile([65, 1024], f32, tag=f"cs{i}", name=f"cs{i}")
                    for i in range(2)]
        flush_idx = 0
        for s in range(NSTEP):
            g, u = s // 2, s % 2
            ps = psum.tile([P, 1024], f32, tag="pair", name="pair")
            al_ap = al_t[:, s * 256:(s + 1) * 256].rearrange(
                "p (two m) -> p two m", two=2)
            for h in range(2):
                ar_ap = ar_t[:, g * 2048 + h * 1024:g * 2048 + (h + 1) * 1024] \
                    .rearrange("p (two n) -> p two n", two=2)
                nc.tensor.matmul(ps[:, h * 512:(h + 1) * 512], al_ap, ar_ap,
                                 start=True, stop=False, perf_mode=DR)
            for h, (l_t, r_t) in enumerate(((lw_t, rw_t), (lg_t, rg_t))):
                for kc in range(2):
                    lap = l_t[:, s * 512 + kc * 256:s * 512 + (kc + 1) * 256] \
                        .rearrange("p (two m) -> p two m", two=2)
                    rap = r_t[:, g * 2048 + kc * 1024:g * 2048 + (kc + 1) * 1024] \
                        .rearrange("p (two n) -> p two n", two=2)
                    nc.tensor.matmul(ps[:, h * 512:(h + 1) * 512], lap, rap,
                                     start=False, stop=(kc == 1), perf_mode=DR)
            tau = taup.tile([P, 1024], f16, tag="tau", name="tau")
            nc.scalar.activation(tau[:], ps[:],
                                 mybir.ActivationFunctionType.Exp,
                                 bias=0.0, scale=scale)
            kp = kpp.tile([P, 1024], f16, tag="kp", name="kp")
            nc.vector._custom_dve(POWSUM5, out=kp[:], in0=tau[:])
            for h in range(2):
                sl = slice(h * 512, (h + 1) * 512)
                dummy = dmp.tile([P, 512], f16, tag="dm", name="dm")
                nc.vector.scalar_tensor_tensor(
                    out=dummy[:], in0=kp[:, sl], scalar=1.0, in1=kp[:, sl],
                    op0=MULT, op1=mybir.AluOpType.min,
                    accum_out=acc_t[:, 3 * s + h:3 * s + h + 1])
            dummy = dmp.tile([P, 512], f16, tag="dm", name="dm")
            nc.vector.scalar_tensor_tensor(
                out=dummy[:], in0=kp[:, 0:512], scalar=1.0, in1=kp[:, 512:1024],
                op0=MULT, op1=MULT, accum_out=acc_t[:, 3 * s + 2:3 * s + 3])
            # column sums: one PSUM row per group, 32-aligned slots
            cs_cur = cs_tiles[(g // 3) % 2]
            q = (g % 3) * 32
            for h in range(2):
                nc.tensor.matmul(cs_cur[q:q + 1, h * 512:(h + 1) * 512],
                                 ones_t[:], kp[:, h * 512:(h + 1) * 512],
                                 start=(u == 0), stop=(u == 1))
            if u == 1 and g in (2, 5, 8):
                nc.scalar.copy(stage[flush_idx][:], cs_cur[:])
                flush_idx += 1
        for i in range(3):
            nc.sync.dma_start(cs_d.ap()[3 * i:3 * i + 3, :],
                              stage[i][0:65:32, :])
        nc.sync.dma_start(acc_d.ap()[:], acc_t[:])
    nc.compile()
    return nc


def _split4(x):
    """4-term fp8 split of x (f64): sum of returned rows ~ x."""
    outs = []
    r = x.copy()
    for _ in range(4):
        h = r.astype(F8)
        outs.append(h)
        r = r - h.astype(np.float64)
    return outs


def _k16_of_tau(tau16):
    """Device-replica: k16 = f16(powsum5_f32(f16 tau)); every consumer
    (rowsum reduce, S1 product, colsum matmul) reads this same value."""
    t = tau16.astype(np.float32)
    t2 = t * t
    t4 = t2 * t2
    t8 = t4 * t4
    k = (((t + t2) + (t4 + t8)) + t8 * t8).astype(np.float16)
    return k.astype(np.float64)


def kernel(W, G, **_):
    import os
    os.environ["BASS_NEVER_TRACE"] = "1"
    from concourse.bass_utils import run_bass_kernel_spmd
    W = np.asarray(W, dtype=np.float32)
    G = np.asarray(G, dtype=np.float32)
    n = W.shape[0]
    N = 2 * n

    # bandwidth from the full-precision inputs (closed form, f64)
    W64, G64 = W.astype(np.float64), G.astype(np.float64)
    sqW_t = (W64 * W64).sum(1)
    sqG_t = (G64 * G64).sum(1)
    colsum = W64.sum(0) + G64.sum(0)
    sum_d2 = 2.0 * N * (sqW_t.sum() + sqG_t.sum()) - 2.0 * (colsum * colsum).sum()
    bw = sum_d2 / (N * N - N) / 4.0
    scale = float(np.float32(1.0 / (8.0 * bw)))

    # fp8 quantization + aug splits (from quantized rows: keeps d2_q >= 0
    # and the diagonal exactly zero pre-rounding)
    W8 = W.astype(F8)
    G8 = G.astype(F8)
    W8f = W8.astype(np.float64)
    G8f = G8.astype(np.float64)
    aW = -0.5 * (W8f * W8f).sum(1)
    aG = -0.5 * (G8f * G8f).sum(1)
    # 4-term fp8 split of a/2 (e4m3 max is 240; |a| can exceed it), applied
    # through selector rows of 2.0 in the aug matmul.
    aW4 = _split4(aW / 2.0)
    aG4 = _split4(aG / 2.0)
    aWs = 2.0 * sum(a.astype(np.float64) for a in aW4)
    aGs = 2.0 * sum(a.astype(np.float64) for a in aG4)
    W8T = np.ascontiguousarray(W8.T)  # [feat, row]
    G8T = np.ascontiguousarray(G8.T)

    scheds = [_schedule(c) for c in range(NCORES)]
    in_maps = []
    for c in range(NCORES):
        st = scheds[c]
        rw = np.zeros((P, NG * 2048), F8)
        rg = np.zeros((P, NG * 2048), F8)
        lw = np.zeros((P, NSTEP * 512), F8)
        lg = np.zeros((P, NSTEP * 512), F8)
        ar = np.zeros((12, NG * 2048), F8)
        al = np.zeros((12, NSTEP * 256), F8)
        for g in range(NG):
            jc = st[2 * g][1]
            cols = slice(jc * 512, (jc + 1) * 512)
            for q in range(4):
                rw[:, g * 2048 + q * 512:g * 2048 + (q + 1) * 512] = \
                    W8T[q * P:(q + 1) * P, cols]
                rg[:, g * 2048 + q * 512:g * 2048 + (q + 1) * 512] = \
                    G8T[q * P:(q + 1) * P, cols]
                # half-major aug rhs: [W: plane0|plane1, G: plane0|plane1]
                ar[q, g * 2048 + 0:g * 2048 + 512] = aW4[q][cols]
                ar[4 + q, g * 2048 + 0:g * 2048 + 512] = 2.0
                ar[q, g * 2048 + 1024:g * 2048 + 1536] = aG4[q][cols]
                ar[8 + q, g * 2048 + 1024:g * 2048 + 1536] = 2.0
        for s, (r, jc, su) in enumerate(st):
            rsl = slice(r * P, (r + 1) * P)
            for kc in range(2):
                for i in range(2):
                    fsl = slice(kc * 256 + i * P, kc * 256 + (i + 1) * P)
                    dst = slice(s * 512 + kc * 256 + i * P,
                                s * 512 + kc * 256 + (i + 1) * P)
                    lw[:, dst] = W8T[fsl, rsl]
                    lg[:, dst] = G8T[fsl, rsl]
            for q in range(4):
                al[q, s * 256:s * 256 + P] = 2.0
                al[4 + q, s * 256:s * 256 + P] = aW4[q][rsl]
                al[8 + q, s * 256:s * 256 + P] = aG4[q][rsl]
        in_maps.append({"rw": rw, "rg": rg, "lw": lw, "lg": lg,
                        "ar": ar, "al": al})

    global LAST_SCALE
    LAST_SCALE = scale
    nc = _build(scale)
    res = run_bass_kernel_spmd(nc, in_maps, core_ids=list(range(NCORES)))
    global LAST_RESULT
    LAST_RESULT = res

    # host combine (f64)
    S1 = 0.0
    sW = np.zeros(n)
    sG = np.zeros(n)
    for c in range(NCORES):
        out = res.results[c]
        acc = out["acc"].astype(np.float64)
        cs = out["cs"].astype(np.float64)
        for s, (r, jc, su) in enumerate(scheds[c]):
            rsl = slice(r * P, (r + 1) * P)
            sW[rsl] += acc[:, 3 * s + 0]
            sG[rsl] += acc[:, 3 * s + 1]
            S1 += (2.0 if su else 1.0) * acc[:, 3 * s + 2].sum()
        for g in range(NG):
            jc, su = scheds[c][2 * g][1], scheds[c][2 * g][2]
            if su:
                csl = slice(jc * 512, (jc + 1) * 512)
                sW[csl] += cs[g, 0:512]
                sG[csl] += cs[g, 512:1024]

    # diagonal: replace device-computed quantized values with exact 5.0
    sc32 = np.float32(scale)
    for X8f, aXs, sX, which in ((W8f, aWs, sW, 0), (G8f, aGs, sG, 1)):
        g_ii = (X8f * X8f).sum(1)
        P_ii = (g_ii + 2.0 * aXs).astype(np.float32)
        tau = (np.exp(P_ii * sc32)).astype(np.float16)
        k16 = _k16_of_tau(tau)
        sX += 5.0 - k16
        if which == 0:
            kWd = k16
        else:
            kGd = k16
    S1 += (25.0 - kWd * kGd).sum()

    T = S1 - (2.0 / n) * (sW * sG).sum() + sW.sum() * sG.sum() / (n * n)
    loss = -T / ((n - 1) ** 2)
    return np.float32(loss)


# revision 19
# speedup vs baseline: 1.8969x; 1.3007x over previous
"""HSIC loss kernel for 8 TRN2 NeuronCores.

Math: loss = -tr(CKW.CKG)/(n-1)^2 with CKX = KX.H, H = I - 1/n.
Expanded:  T = S1 - (2/n) sum_i sW_i sG_i + SW SG/n^2, loss = -T/(n-1)^2
where S1 = sum_ij KW.KG, sX = row sums of KX (KX symmetric).

Symmetry: only the region R = {(i,j): j >= 512*floor(i/512)} of each 4096^2
kernel block is computed (144 [128,512] tiles per matrix instead of 256).
For elements below R, the mirror (strictly-upper 512-blocks) supplies them:
S1 doubles those tiles' contributions, and row sums get the mirrored part
from COLUMN sums of the computed tiles (ones-vector matmuls into PSUM).

Sharding: [128,512]-tile-rows r=0..31; core c owns rows {2c,2c+1,30-2c,31-2c}
= 18 (r,chunk) pairs/core, a perfectly balanced split. The SPMD program is
IDENTICAL on every core: 18 uniform steps; all per-core variation is data
(lhsT/rhs/aug streams staged in compute order by the host).

Per step (one (r,jc) pair, W and G halves side by side in PSUM [128,1024]):
fp8(e4m3) DoubleRow matmuls: 1 aug matmul (K=12: 4-way fp8 splits of
a_j = -sq_j/2 for both halves + per-row a_i via ones-selector rows) then
2 DR matmuls per half (K=256 each) accumulate the dot products. ACT does one
pair-wide Exp -> tau (f16). DVE: custom POWSUM4 (s = t^2+t^4+t^8+t^16), then
per half a 4x-mode scalar_tensor_tensor k = tau + s with accum = rowsum(k),
then one 4x STT kW*kG with accum -> S1 partials. A ones[128,1] matmul per
group accumulates column sums into per-group PSUM rows (partition 32-aligned
slots, flushed by 3 ACT copies). Host combines everything in f64 and replaces
the (quantized) diagonal with its exact value.
"""
import numpy as np
import ml_dtypes
from contextlib import ExitStack

import concourse.bass as bass
import concourse.tile as tile
from concourse import bacc, mybir
import concourse.dve_ops as dve_ops
from concourse.dve_spec import Spec, Src0, lower, _has_src1
from concourse.dve_ops import DveOp
from concourse.dve_uop import DveOpSpec

N_ROWS = 4096
D = 512
NCORES = 8
P = 128
NSTEP = 18
NG = 9
F8 = ml_dtypes.float8_e4m3
LAST_RESULT = None
LAST_SCALE = None

f32 = mybir.dt.float32
f16 = mybir.dt.float16
f8e4 = mybir.dt.float8e4
DR = mybir.MatmulPerfMode.DoubleRow
ADD = mybir.AluOpType.add
MULT = mybir.AluOpType.mult


def _ref_powsum5(in0, in1, s0, s1, imm2):
    t = in0.astype(np.float32)
    t2 = t * t
    t4 = t2 * t2
    t8 = t4 * t4
    return (((t + t2) + (t4 + t8)) + t8 * t8).astype(np.float32)


def _register_powsum5():
    name = "POWSUM5_HSIC_ANT"
    for op in dve_ops.OPS:
        if op.name == name:
            return op
    t = Src0
    t2 = t * t
    t4 = t2 * t2
    t8 = t4 * t4
    spec = Spec(body=((t + t2) + (t4 + t8)) + t8 * t8, reference=_ref_powsum5)
    shas = {}
    for ver in ("v3", "v4"):
        tmp = DveOpSpec(name=name, opcode=1, uops=lower(spec, ver=ver),
                        rd1_en=_has_src1(spec))
        shas[ver] = tmp.sha(ver)
    op = DveOp(name, spec, subdim=False, uops_sha=shas)
    dve_ops.OPS.append(op)
    dve_ops._SUB_OPCODE_FOR_NAME[name] = (
        dve_ops._CUSTOM_DVE_ROW_BASE + len(dve_ops.OPS) - 1)
    dve_ops.CUSTOM_DVE_SPECS[name] = op.spec
    return op


def _schedule(c):
    """18 (tile_row, chunk, strict_upper) steps for core c, chunk-major.
    Consecutive step pairs (groups) always share the chunk."""
    rows = [2 * c, 2 * c + 1, 30 - 2 * c, 31 - 2 * c]
    steps = []
    for jc in range(8):
        for r in rows:
            if jc >= r // 4:
                steps.append((r, jc, jc > r // 4))
    assert len(steps) == NSTEP
    for g in range(NG):
        assert steps[2 * g][1] == steps[2 * g + 1][1]
        assert steps[2 * g][2] == steps[2 * g + 1][2]
    return steps


def _build(scale: float):
    POWSUM5 = _register_powsum5()
    nc = bacc.Bacc("TRN2", target_bir_lowering=False, debug=False)

    rw_d = nc.dram_tensor("rw", [P, NG * 2048], f8e4, kind="ExternalInput")
    rg_d = nc.dram_tensor("rg", [P, NG * 2048], f8e4, kind="ExternalInput")
    lw_d = nc.dram_tensor("lw", [P, NSTEP * 512], f8e4, kind="ExternalInput")
    lg_d = nc.dram_tensor("lg", [P, NSTEP * 512], f8e4, kind="ExternalInput")
    ar_d = nc.dram_tensor("ar", [12, NG * 2048], f8e4, kind="ExternalInput")
    al_d = nc.dram_tensor("al", [12, NSTEP * 256], f8e4, kind="ExternalInput")
    acc_d = nc.dram_tensor("acc", [P, 3 * NSTEP], f32, kind="ExternalOutput")
    cs_d = nc.dram_tensor("cs", [12, 1024], f32, kind="ExternalOutput")

    with tile.TileContext(nc) as tc, ExitStack() as ctx:
        const = ctx.enter_context(tc.tile_pool(name="const", bufs=1))
        psum = ctx.enter_context(tc.tile_pool(name="psum", bufs=2, space="PSUM"))
        csp = ctx.enter_context(tc.tile_pool(name="csp", bufs=1, space="PSUM"))
        taup = ctx.enter_context(tc.tile_pool(name="taup", bufs=2))
        kpp = ctx.enter_context(tc.tile_pool(name="kpp", bufs=2))
        dmp = ctx.enter_context(tc.tile_pool(name="dmp", bufs=2))

        rw_t = const.tile([P, NG * 2048], f8e4, tag="rw", name="rw_t")
        rg_t = const.tile([P, NG * 2048], f8e4, tag="rg", name="rg_t")
        lw_t = const.tile([P, NSTEP * 512], f8e4, tag="lw", name="lw_t")
        lg_t = const.tile([P, NSTEP * 512], f8e4, tag="lg", name="lg_t")
        ar_t = const.tile([12, NG * 2048], f8e4, tag="ar", name="ar_t")
        al_t = const.tile([12, NSTEP * 256], f8e4, tag="al", name="al_t")
        ones_t = const.tile([P, 1], f16, tag="ones", name="ones_t")
        acc_t = const.tile([P, 3 * NSTEP], f32, tag="acc", name="acc_t")
        stage = [const.tile([65, 1024], f32, tag=f"st{i}", name=f"st{i}")
                 for i in range(3)]
        nc.vector.memset(ones_t[:], 1.0)
        for i in range(3):
            gs, ge = 3 * i, 3 * (i + 1)
            nc.sync.dma_start(ar_t[:, gs * 2048:ge * 2048],
                              ar_d.ap()[:, gs * 2048:ge * 2048])
            nc.sync.dma_start(al_t[:, gs * 512:ge * 512],
                              al_d.ap()[:, gs * 512:ge * 512])
            nc.sync.dma_start(rw_t[:, gs * 2048:ge * 2048],
                              rw_d.ap()[:, gs * 2048:ge * 2048])
            nc.sync.dma_start(rg_t[:, gs * 2048:ge * 2048],
                              rg_d.ap()[:, gs * 2048:ge * 2048])
            nc.sync.dma_start(lw_t[:, gs * 1024:ge * 1024],
                              lw_d.ap()[:, gs * 1024:ge * 1024])
            nc.sync.dma_start(lg_t[:, gs * 1024:ge * 1024],
                              lg_d.ap()[:, gs * 1024:ge * 1024])

        cs_tiles = [csp.tile([65, 1024], f32, tag=f"cs{i}", name=f"cs{i}")
                    for i in range(2)]
        flush_idx = 0
        for s in range(NSTEP):
            g, u = s // 2, s % 2
            ps = psum.tile([P, 1024], f32, tag="pair", name="pair")
            al_ap = al_t[:, s * 256:(s + 1) * 256].rearrange(
                "p (two m) -> p two m", two=2)
            for h in range(2):
                ar_ap = ar_t[:, g * 2048 + h * 1024:g * 2048 + (h + 1) * 1024] \
                    .rearrange("p (two n) -> p two n", two=2)
                nc.tensor.matmul(ps[:, h * 512:(h + 1) * 512], al_ap, ar_ap,
                                 start=True, stop=False, perf_mode=DR)
            for h, (l_t, r_t) in enumerate(((lw_t, rw_t), (lg_t, rg_t))):
                for kc in range(2):
                    lap = l_t[:, s * 512 + kc * 256:s * 512 + (kc + 1) * 256] \
                        .rearrange("p (two m) -> p two m", two=2)
                    rap = r_t[:, g * 2048 + kc * 1024:g * 2048 + (kc + 1) * 1024] \
                        .rearrange("p (two n) -> p two n", two=2)
                    nc.tensor.matmul(ps[:, h * 512:(h + 1) * 512], lap, rap,
                                     start=False, stop=(kc == 1), perf_mode=DR)
            tau = taup.tile([P, 1024], f16, tag="tau", name="tau")
            nc.scalar.activation(tau[:], ps[:],
                                 mybir.ActivationFunctionType.Exp,
                                 bias=0.0, scale=scale)
            kp = kpp.tile([P, 1024], f16, tag="kp", name="kp")
            nc.vector._custom_dve(POWSUM5, out=kp[:], in0=tau[:])
            for h in range(2):
                sl = slice(h * 512, (h + 1) * 512)
                dummy = dmp.tile([P, 512], f16, tag="dm", name="dm")
                nc.vector.tensor_scalar(
                    out=dummy[:], in0=kp[:, sl], scalar1=1.0, scalar2=0.0,
                    op0=MULT, op1=ADD,
                    accum_out=acc_t[:, 3 * s + h:3 * s + h + 1])
            prod = dmp.tile([P, 512], f16, tag="prod", name="prod")
            nc.gpsimd.tensor_tensor(out=prod[:], in0=kp[:, 0:512],
                                    in1=kp[:, 512:1024], op=MULT)
            dummy = dmp.tile([P, 512], f16, tag="dm", name="dm")
            nc.vector.tensor_scalar(
                out=dummy[:], in0=prod[:], scalar1=1.0, scalar2=0.0,
                op0=MULT, op1=ADD,
                accum_out=acc_t[:, 3 * s + 2:3 * s + 3])
            # column sums: one PSUM row per group, 32-aligned slots
            cs_cur = cs_tiles[(g // 3) % 2]
            q = (g % 3) * 32
            for h in range(2):
                nc.tensor.matmul(cs_cur[q:q + 1, h * 512:(h + 1) * 512],
                                 ones_t[:], kp[:, h * 512:(h + 1) * 512],
                                 start=(u == 0), stop=(u == 1))
            if u == 1 and g in (2, 5, 8):
                nc.scalar.copy(stage[flush_idx][:], cs_cur[:])
                flush_idx += 1
        for i in range(3):
            nc.sync.dma_start(cs_d.ap()[3 * i:3 * i + 3, :],
                              stage[i][0:65:32, :])
        nc.sync.dma_start(acc_d.ap()[:], acc_t[:])
    nc.compile()
    return nc


def _split4(x):
    """4-term fp8 split of x (f64): sum of returned rows ~ x."""
    outs = []
    r = x.copy()
    for _ in range(4):
        h = r.astype(F8)
        outs.append(h)
        r = r - h.astype(np.float64)
    return outs


def _k16_of_tau(tau16):
    """Device-replica: k16 = f16(powsum5_f32(f16 tau)); every consumer
    (rowsum reduce, S1 product, colsum matmul) reads this same value."""
    t = tau16.astype(np.float32)
    t2 = t * t
    t4 = t2 * t2
    t8 = t4 * t4
    k = (((t + t2) + (t4 + t8)) + t8 * t8).astype(np.float16)
    return k.astype(np.float64)


def kernel(W, G, **_):
    import os
    os.environ["BASS_NEVER_TRACE"] = "1"
    from concourse.bass_utils import run_bass_kernel_spmd
    W = np.asarray(W, dtype=np.float32)
    G = np.asarray(G, dtype=np.float32)
    n = W.shape[0]
    N = 2 * n

    # bandwidth from the full-precision inputs (closed form, f64)
    W64, G64 = W.astype(np.float64), G.astype(np.float64)
    sqW_t = (W64 * W64).sum(1)
    sqG_t = (G64 * G64).sum(1)
    colsum = W64.sum(0) + G64.sum(0)
    sum_d2 = 2.0 * N * (sqW_t.sum() + sqG_t.sum()) - 2.0 * (colsum * colsum).sum()
    bw = sum_d2 / (N * N - N) / 4.0
    scale = float(np.float32(1.0 / (8.0 * bw)))

    # fp8 quantization + aug splits (from quantized rows: keeps d2_q >= 0
    # and the diagonal exactly zero pre-rounding)
    W8 = W.astype(F8)
    G8 = G.astype(F8)
    W8f = W8.astype(np.float64)
    G8f = G8.astype(np.float64)
    aW = -0.5 * (W8f * W8f).sum(1)
    aG = -0.5 * (G8f * G8f).sum(1)
    # 4-term fp8 split of a/2 (e4m3 max is 240; |a| can exceed it), applied
    # through selector rows of 2.0 in the aug matmul.
    aW4 = _split4(aW / 2.0)
    aG4 = _split4(aG / 2.0)
    aWs = 2.0 * sum(a.astype(np.float64) for a in aW4)
    aGs = 2.0 * sum(a.astype(np.float64) for a in aG4)
    W8T = np.ascontiguousarray(W8.T)  # [feat, row]
    G8T = np.ascontiguousarray(G8.T)

    scheds = [_schedule(c) for c in range(NCORES)]
    in_maps = []
    for c in range(NCORES):
        st = scheds[c]
        rw = np.zeros((P, NG * 2048), F8)
        rg = np.zeros((P, NG * 2048), F8)
        lw = np.zeros((P, NSTEP * 512), F8)
        lg = np.zeros((P, NSTEP * 512), F8)
        ar = np.zeros((12, NG * 2048), F8)
        al = np.zeros((12, NSTEP * 256), F8)
        for g in range(NG):
            jc = st[2 * g][1]
            cols = slice(jc * 512, (jc + 1) * 512)
            for q in range(4):
                rw[:, g * 2048 + q * 512:g * 2048 + (q + 1) * 512] = \
                    W8T[q * P:(q + 1) * P, cols]
                rg[:, g * 2048 + q * 512:g * 2048 + (q + 1) * 512] = \
                    G8T[q * P:(q + 1) * P, cols]
                # half-major aug rhs: [W: plane0|plane1, G: plane0|plane1]
                ar[q, g * 2048 + 0:g * 2048 + 512] = aW4[q][cols]
                ar[4 + q, g * 2048 + 0:g * 2048 + 512] = 2.0
                ar[q, g * 2048 + 1024:g * 2048 + 1536] = aG4[q][cols]
                ar[8 + q, g * 2048 + 1024:g * 2048 + 1536] = 2.0
        for s, (r, jc, su) in enumerate(st):
            rsl = slice(r * P, (r + 1) * P)
            for kc in range(2):
                for i in range(2):
                    fsl = slice(kc * 256 + i * P, kc * 256 + (i + 1) * P)
                    dst = slice(s * 512 + kc * 256 + i * P,
                                s * 512 + kc * 256 + (i + 1) * P)
                    lw[:, dst] = W8T[fsl, rsl]
                    lg[:, dst] = G8T[fsl, rsl]
            for q in range(4):
                al[q, s * 256:s * 256 + P] = 2.0
                al[4 + q, s * 256:s * 256 + P] = aW4[q][rsl]
                al[8 + q, s * 256:s * 256 + P] = aG4[q][rsl]
        in_maps.append({"rw": rw, "rg": rg, "lw": lw, "lg": lg,
                        "ar": ar, "al": al})

    global LAST_SCALE
    LAST_SCALE = scale
    nc = _build(scale)
    res = run_bass_kernel_spmd(nc, in_maps, core_ids=list(range(NCORES)))
    global LAST_RESULT
    LAST_RESULT = res

    # host combine (f64)
    S1 = 0.0
    sW = np.zeros(n)
    sG = np.zeros(n)
    for c in range(NCORES):
        out = res.results[c]
        acc = out["acc"].astype(np.float64)
        cs = out["cs"].astype(np.float64)
        for s, (r, jc, su) in enumerate(scheds[c]):
            rsl = slice(r * P, (r + 1) * P)
            sW[rsl] += acc[:, 3 * s + 0]
            sG[rsl] += acc[:, 3 * s + 1]
            S1 += (2.0 if su else 1.0) * acc[:, 3 * s + 2].sum()
        for g in range(NG):
            jc, su = scheds[c][2 * g][1], scheds[c][2 * g][2]
            if su:
                csl = slice(jc * 512, (jc + 1) * 512)
                sW[csl] += cs[g, 0:512]
                sG[csl] += cs[g, 512:1024]

    # diagonal: replace device-computed quantized values with exact 5.0
    sc32 = np.float32(scale)
    for X8f, aXs, sX, which in ((W8f, aWs, sW, 0), (G8f, aGs, sG, 1)):
        g_ii = (X8f * X8f).sum(1)
        P_ii = (g_ii + 2.0 * aXs).astype(np.float32)
        tau = (np.exp(P_ii * sc32)).astype(np.float16)
        k16 = _k16_of_tau(tau)
        sX += 5.0 - k16
        if which == 0:
            kWd = k16
        else:
            kGd = k16
    S1 += (25.0 - kWd * kGd).sum()

    T = S1 - (2.0 / n) * (sW * sG).sum() + sW.sum() * sG.sum() / (n * n)
    loss = -T / ((n - 1) ** 2)
    return np.float32(loss)


# revision 22
# speedup vs baseline: 2.1027x; 1.1085x over previous
"""HSIC loss kernel for 8 TRN2 NeuronCores.

Math: loss = -tr(CKW.CKG)/(n-1)^2 with CKX = KX.H, H = I - 1/n.
Expanded:  T = S1 - (2/n) sum_i sW_i sG_i + SW SG/n^2, loss = -T/(n-1)^2
where S1 = sum_ij KW.KG, sX = row sums of KX (KX symmetric).

Symmetry: only the region R = {(i,j): j >= 512*floor(i/512)} of each 4096^2
kernel block is computed (144 [128,512] tiles per matrix instead of 256).
For elements below R, the mirror (strictly-upper 512-blocks) supplies them:
S1 doubles those tiles' contributions, and row sums get the mirrored part
from COLUMN sums of the computed tiles (ones-vector matmuls into PSUM).

Sharding: [128,512]-tile-rows r=0..31; core c owns rows {2c,2c+1,30-2c,31-2c}
= 18 (r,chunk) pairs/core, a perfectly balanced split. The SPMD program is
IDENTICAL on every core: 18 uniform steps; all per-core variation is data
(lhsT/rhs/aug streams staged in compute order by the host).

Per step (one (r,jc) pair, W and G halves side by side in PSUM [128,1024]):
fp8(e4m3) DoubleRow matmuls: 1 aug matmul (K=12: 4-way fp8 splits of
a_j = -sq_j/2 for both halves + per-row a_i via ones-selector rows) then
2 DR matmuls per half (K=256 each) accumulate the dot products. ACT does one
pair-wide Exp -> tau (f16). DVE: custom POWSUM4 (s = t^2+t^4+t^8+t^16), then
per half a 4x-mode scalar_tensor_tensor k = tau + s with accum = rowsum(k),
then one 4x STT kW*kG with accum -> S1 partials. A ones[128,1] matmul per
group accumulates column sums into per-group PSUM rows (partition 32-aligned
slots, flushed by 3 ACT copies). Host combines everything in f64 and replaces
the (quantized) diagonal with its exact value.
"""
import numpy as np
import ml_dtypes
from contextlib import ExitStack

import concourse.bass as bass
import concourse.tile as tile
from concourse import bacc, mybir
import concourse.dve_ops as dve_ops
from concourse.dve_spec import Spec, Src0, lower, _has_src1
from concourse.dve_ops import DveOp
from concourse.dve_uop import DveOpSpec

N_ROWS = 4096
D = 512
NCORES = 8
P = 128
NSTEP = 18
NG = 9
F8 = ml_dtypes.float8_e4m3
LAST_RESULT = None
LAST_SCALE = None

f32 = mybir.dt.float32
f16 = mybir.dt.float16
f8e4 = mybir.dt.float8e4
DR = mybir.MatmulPerfMode.DoubleRow
ADD = mybir.AluOpType.add
MULT = mybir.AluOpType.mult


def _ref_powsum5(in0, in1, s0, s1, imm2):
    t = in0.astype(np.float32)
    t2 = t * t
    t4 = t2 * t2
    t8 = t4 * t4
    return (((t + t2) + (t4 + t8)) + t8 * t8).astype(np.float32)


def _register_powsum5():
    name = "POWSUM5_HSIC_ANT"
    for op in dve_ops.OPS:
        if op.name == name:
            return op
    t = Src0
    t2 = t * t
    t4 = t2 * t2
    t8 = t4 * t4
    spec = Spec(body=((t + t2) + (t4 + t8)) + t8 * t8, reference=_ref_powsum5)
    shas = {}
    for ver in ("v3", "v4"):
        tmp = DveOpSpec(name=name, opcode=1, uops=lower(spec, ver=ver),
                        rd1_en=_has_src1(spec))
        shas[ver] = tmp.sha(ver)
    op = DveOp(name, spec, subdim=False, uops_sha=shas)
    dve_ops.OPS.append(op)
    dve_ops._SUB_OPCODE_FOR_NAME[name] = (
        dve_ops._CUSTOM_DVE_ROW_BASE + len(dve_ops.OPS) - 1)
    dve_ops.CUSTOM_DVE_SPECS[name] = op.spec
    return op


def _schedule(c):
    """18 (tile_row, chunk, strict_upper) steps for core c. The 4 straddle
    pairs (jc == block row: counted once, no colsum mirror) come FIRST as
    groups 0-1, then the 14 strict-upper pairs chunk-major as groups 2-8.
    This fixed straddle/upper step layout is identical on every core, so the
    two S1 PSUM accumulators can be routed by step index in the uniform
    SPMD program. Groups (consecutive step pairs) always share the chunk."""
    rows = [2 * c, 2 * c + 1, 30 - 2 * c, 31 - 2 * c]
    straddle = []
    upper = []
    for jc in range(8):
        for r in rows:
            if jc == r // 4:
                straddle.append((r, jc, False))
            elif jc > r // 4:
                upper.append((r, jc, True))
    steps = straddle + upper
    assert len(straddle) == 4 and len(steps) == NSTEP
    for g in range(NG):
        assert steps[2 * g][1] == steps[2 * g + 1][1]
        assert steps[2 * g][2] == steps[2 * g + 1][2]
    return steps


def _build(scale: float):
    POWSUM5 = _register_powsum5()
    nc = bacc.Bacc("TRN2", target_bir_lowering=False, debug=False)

    rw_d = nc.dram_tensor("rw", [P, NG * 2048], f8e4, kind="ExternalInput")
    rg_d = nc.dram_tensor("rg", [P, NG * 2048], f8e4, kind="ExternalInput")
    lw_d = nc.dram_tensor("lw", [P, NSTEP * 512], f8e4, kind="ExternalInput")
    lg_d = nc.dram_tensor("lg", [P, NSTEP * 512], f8e4, kind="ExternalInput")
    ar_d = nc.dram_tensor("ar", [12, NG * 2048], f8e4, kind="ExternalInput")
    al_d = nc.dram_tensor("al", [12, NSTEP * 256], f8e4, kind="ExternalInput")
    id_d = nc.dram_tensor("ident", [P, 128], f16, kind="ExternalInput")
    acc_d = nc.dram_tensor("acc", [P, 2 * NSTEP + 2], f32, kind="ExternalOutput")
    cs_d = nc.dram_tensor("cs", [12, 1024], f32, kind="ExternalOutput")

    with tile.TileContext(nc) as tc, ExitStack() as ctx:
        const = ctx.enter_context(tc.tile_pool(name="const", bufs=1))
        psum = ctx.enter_context(tc.tile_pool(name="psum", bufs=2, space="PSUM"))
        csp = ctx.enter_context(tc.tile_pool(name="csp", bufs=1, space="PSUM"))
        taup = ctx.enter_context(tc.tile_pool(name="taup", bufs=3))
        kpp = ctx.enter_context(tc.tile_pool(name="kpp", bufs=4))
        dmp = ctx.enter_context(tc.tile_pool(name="dmp", bufs=4))

        rw_t = const.tile([P, NG * 2048], f8e4, tag="rw", name="rw_t")
        rg_t = const.tile([P, NG * 2048], f8e4, tag="rg", name="rg_t")
        lw_t = const.tile([P, NSTEP * 512], f8e4, tag="lw", name="lw_t")
        lg_t = const.tile([P, NSTEP * 512], f8e4, tag="lg", name="lg_t")
        ar_t = const.tile([12, NG * 2048], f8e4, tag="ar", name="ar_t")
        al_t = const.tile([12, NSTEP * 256], f8e4, tag="al", name="al_t")
        ones_t = const.tile([P, 1], f16, tag="ones", name="ones_t")
        acc_t = const.tile([P, 2 * NSTEP + 2], f32, tag="acc", name="acc_t")
        ident_t = const.tile([P, 128], f16, tag="ident", name="ident_t")
        stage = [const.tile([65, 1024], f32, tag=f"st{i}", name=f"st{i}")
                 for i in range(3)]
        nc.vector.memset(ones_t[:], 1.0)
        nc.sync.dma_start(ident_t[:], id_d.ap()[:])
        for i in range(3):
            gs, ge = 3 * i, 3 * (i + 1)
            nc.sync.dma_start(ar_t[:, gs * 2048:ge * 2048],
                              ar_d.ap()[:, gs * 2048:ge * 2048])
            nc.sync.dma_start(al_t[:, gs * 512:ge * 512],
                              al_d.ap()[:, gs * 512:ge * 512])
            nc.sync.dma_start(rw_t[:, gs * 2048:ge * 2048],
                              rw_d.ap()[:, gs * 2048:ge * 2048])
            nc.sync.dma_start(rg_t[:, gs * 2048:ge * 2048],
                              rg_d.ap()[:, gs * 2048:ge * 2048])
            nc.sync.dma_start(lw_t[:, gs * 1024:ge * 1024],
                              lw_d.ap()[:, gs * 1024:ge * 1024])
            nc.sync.dma_start(lg_t[:, gs * 1024:ge * 1024],
                              lg_d.ap()[:, gs * 1024:ge * 1024])

        cs_tile = csp.tile([65, 1024], f32, tag="cs0", name="cs0")
        s1_str = csp.tile([P, 512], f32, tag="s1a", name="s1a")
        s1_upp = csp.tile([P, 512], f32, tag="s1b", name="s1b")
        LAG = 2
        kp_list = {}
        flush_state = [0]

        def emit_step(s):
            g, u = s // 2, s % 2
            ps = psum.tile([P, 1024], f32, tag="pair", name="pair")
            al_ap = al_t[:, s * 256:(s + 1) * 256].rearrange(
                "p (two m) -> p two m", two=2)
            for h in range(2):
                ar_ap = ar_t[:, g * 2048 + h * 1024:g * 2048 + (h + 1) * 1024] \
                    .rearrange("p (two n) -> p two n", two=2)
                nc.tensor.matmul(ps[:, h * 512:(h + 1) * 512], al_ap, ar_ap,
                                 start=True, stop=False, perf_mode=DR)
            for h, (l_t, r_t) in enumerate(((lw_t, rw_t), (lg_t, rg_t))):
                for kc in range(2):
                    lap = l_t[:, s * 512 + kc * 256:s * 512 + (kc + 1) * 256] \
                        .rearrange("p (two m) -> p two m", two=2)
                    rap = r_t[:, g * 2048 + kc * 1024:g * 2048 + (kc + 1) * 1024] \
                        .rearrange("p (two n) -> p two n", two=2)
                    nc.tensor.matmul(ps[:, h * 512:(h + 1) * 512], lap, rap,
                                     start=False, stop=(kc == 1), perf_mode=DR)
            tau = taup.tile([P, 1024], f16, tag="tau", name="tau")
            nc.scalar.activation(tau[:], ps[:],
                                 mybir.ActivationFunctionType.Exp,
                                 bias=0.0, scale=scale)
            kp = kpp.tile([P, 1024], f16, tag="kp", name="kp")
            nc.vector._custom_dve(POWSUM5, out=kp[:], in0=tau[:])
            for h in range(2):
                sl = slice(h * 512, (h + 1) * 512)
                dummy = dmp.tile([P, 512], f16, tag="dm", name="dm")
                nc.vector.tensor_scalar(
                    out=dummy[:], in0=kp[:, sl], scalar1=1.0, scalar2=0.0,
                    op0=MULT, op1=ADD,
                    accum_out=acc_t[:, 2 * s + h:2 * s + h + 1])
            kp_list[s] = kp

        def emit_lagged(s):
            g, u = s // 2, s % 2
            kp = kp_list.pop(s)
            # S1: accumulate kW_q^T . kG_q; its diagonal sums to sum(kW*kG).
            # Straddle steps (0-3) and strict-upper steps (4-17) use separate
            # accumulators (host weights them 1x / 2x).
            accT = s1_str if s < 4 else s1_upp
            first = (s == 0) if s < 4 else (s == 4)
            last = (s == 3) if s < 4 else (s == NSTEP - 1)
            for q in range(4):
                nc.tensor.matmul(accT[:, 0:128],
                                 kp[:, q * 128:(q + 1) * 128],
                                 kp[:, 512 + q * 128:512 + (q + 1) * 128],
                                 start=(first and q == 0), stop=(last and q == 3),
                                 skip_group_check=True)
            # column sums into per-group PSUM row (32-aligned slot)
            qrow = (g % 3) * 32
            for h in range(2):
                nc.tensor.matmul(cs_tile[qrow:qrow + 1, h * 512:(h + 1) * 512],
                                 ones_t[:], kp[:, h * 512:(h + 1) * 512],
                                 start=(u == 0), stop=(u == 1),
                                 skip_group_check=True)
            if u == 1 and g in (2, 5, 8):
                nc.scalar.copy(stage[flush_state[0]][:], cs_tile[:])
                flush_state[0] += 1

        for s in range(NSTEP):
            emit_step(s)
            if s >= LAG:
                emit_lagged(s - LAG)
        for s in range(NSTEP - LAG, NSTEP):
            emit_lagged(s)
        for i, accT in enumerate((s1_str, s1_upp)):
            ddump = dmp.tile([P, 128], f32, tag="dd", name="dd")
            nc.vector.scalar_tensor_tensor(
                out=ddump[:], in0=accT[:, 0:128], scalar=1.0, in1=ident_t[:],
                op0=MULT, op1=MULT,
                accum_out=acc_t[:, 2 * NSTEP + i:2 * NSTEP + i + 1])
        for i in range(3):
            nc.sync.dma_start(cs_d.ap()[3 * i:3 * i + 3, :],
                              stage[i][0:65:32, :])
        nc.sync.dma_start(acc_d.ap()[:], acc_t[:])
    nc.compile()
    return nc


def _split4(x):
    """4-term fp8 split of x (f64): sum of returned rows ~ x."""
    outs = []
    r = x.copy()
    for _ in range(4):
        h = r.astype(F8)
        outs.append(h)
        r = r - h.astype(np.float64)
    return outs


def _k16_of_tau(tau16):
    """Device-replica: k16 = f16(powsum5_f32(f16 tau)); every consumer
    (rowsum reduce, S1 product, colsum matmul) reads this same value."""
    t = tau16.astype(np.float32)
    t2 = t * t
    t4 = t2 * t2
    t8 = t4 * t4
    k = (((t + t2) + (t4 + t8)) + t8 * t8).astype(np.float16)
    return k.astype(np.float64)


def kernel(W, G, **_):
    import os
    os.environ["BASS_NEVER_TRACE"] = "1"
    from concourse.bass_utils import run_bass_kernel_spmd
    W = np.asarray(W, dtype=np.float32)
    G = np.asarray(G, dtype=np.float32)
    n = W.shape[0]
    N = 2 * n

    # bandwidth from the full-precision inputs (closed form, f64)
    W64, G64 = W.astype(np.float64), G.astype(np.float64)
    sqW_t = (W64 * W64).sum(1)
    sqG_t = (G64 * G64).sum(1)
    colsum = W64.sum(0) + G64.sum(0)
    sum_d2 = 2.0 * N * (sqW_t.sum() + sqG_t.sum()) - 2.0 * (colsum * colsum).sum()
    bw = sum_d2 / (N * N - N) / 4.0
    scale = float(np.float32(1.0 / (8.0 * bw)))

    # fp8 quantization + aug splits (from quantized rows: keeps d2_q >= 0
    # and the diagonal exactly zero pre-rounding)
    W8 = W.astype(F8)
    G8 = G.astype(F8)
    W8f = W8.astype(np.float64)
    G8f = G8.astype(np.float64)
    aW = -0.5 * (W8f * W8f).sum(1)
    aG = -0.5 * (G8f * G8f).sum(1)
    # 4-term fp8 split of a/2 (e4m3 max is 240; |a| can exceed it), applied
    # through selector rows of 2.0 in the aug matmul.
    aW4 = _split4(aW / 2.0)
    aG4 = _split4(aG / 2.0)
    aWs = 2.0 * sum(a.astype(np.float64) for a in aW4)
    aGs = 2.0 * sum(a.astype(np.float64) for a in aG4)
    W8T = np.ascontiguousarray(W8.T)  # [feat, row]
    G8T = np.ascontiguousarray(G8.T)

    scheds = [_schedule(c) for c in range(NCORES)]
    in_maps = []
    for c in range(NCORES):
        st = scheds[c]
        rw = np.zeros((P, NG * 2048), F8)
        rg = np.zeros((P, NG * 2048), F8)
        lw = np.zeros((P, NSTEP * 512), F8)
        lg = np.zeros((P, NSTEP * 512), F8)
        ar = np.zeros((12, NG * 2048), F8)
        al = np.zeros((12, NSTEP * 256), F8)
        for g in range(NG):
            jc = st[2 * g][1]
            cols = slice(jc * 512, (jc + 1) * 512)
            for q in range(4):
                rw[:, g * 2048 + q * 512:g * 2048 + (q + 1) * 512] = \
                    W8T[q * P:(q + 1) * P, cols]
                rg[:, g * 2048 + q * 512:g * 2048 + (q + 1) * 512] = \
                    G8T[q * P:(q + 1) * P, cols]
                # half-major aug rhs: [W: plane0|plane1, G: plane0|plane1]
                ar[q, g * 2048 + 0:g * 2048 + 512] = aW4[q][cols]
                ar[4 + q, g * 2048 + 0:g * 2048 + 512] = 2.0
                ar[q, g * 2048 + 1024:g * 2048 + 1536] = aG4[q][cols]
                ar[8 + q, g * 2048 + 1024:g * 2048 + 1536] = 2.0
        for s, (r, jc, su) in enumerate(st):
            rsl = slice(r * P, (r + 1) * P)
            for kc in range(2):
                for i in range(2):
                    fsl = slice(kc * 256 + i * P, kc * 256 + (i + 1) * P)
                    dst = slice(s * 512 + kc * 256 + i * P,
                                s * 512 + kc * 256 + (i + 1) * P)
                    lw[:, dst] = W8T[fsl, rsl]
                    lg[:, dst] = G8T[fsl, rsl]
            for q in range(4):
                al[q, s * 256:s * 256 + P] = 2.0
                al[4 + q, s * 256:s * 256 + P] = aW4[q][rsl]
                al[8 + q, s * 256:s * 256 + P] = aG4[q][rsl]
        in_maps.append({"rw": rw, "rg": rg, "lw": lw, "lg": lg,
                        "ar": ar, "al": al,
                        "ident": np.eye(P, dtype=np.float16)})

    global LAST_SCALE
    LAST_SCALE = scale
    nc = _build(scale)
    res = run_bass_kernel_spmd(nc, in_maps, core_ids=list(range(NCORES)))
    global LAST_RESULT
    LAST_RESULT = res

    # host combine (f64)
    S1 = 0.0
    sW = np.zeros(n)
    sG = np.zeros(n)
    for c in range(NCORES):
        out = res.results[c]
        acc = out["acc"].astype(np.float64)
        cs = out["cs"].astype(np.float64)
        for s, (r, jc, su) in enumerate(scheds[c]):
            rsl = slice(r * P, (r + 1) * P)
            sW[rsl] += acc[:, 2 * s + 0]
            sG[rsl] += acc[:, 2 * s + 1]
        S1 += acc[:, 2 * NSTEP].sum() + 2.0 * acc[:, 2 * NSTEP + 1].sum()
        for g in range(NG):
            jc, su = scheds[c][2 * g][1], scheds[c][2 * g][2]
            if su:
                csl = slice(jc * 512, (jc + 1) * 512)
                sW[csl] += cs[g, 0:512]
                sG[csl] += cs[g, 512:1024]

    # diagonal: replace device-computed quantized values with exact 5.0
    sc32 = np.float32(scale)
    for X8f, aXs, sX, which in ((W8f, aWs, sW, 0), (G8f, aGs, sG, 1)):
        g_ii = (X8f * X8f).sum(1)
        P_ii = (g_ii + 2.0 * aXs).astype(np.float32)
        tau = (np.exp(P_ii * sc32)).astype(np.float16)
        k16 = _k16_of_tau(tau)
        sX += 5.0 - k16
        if which == 0:
            kWd = k16
        else:
            kGd = k16
    S1 += (25.0 - kWd * kGd).sum()

    T = S1 - (2.0 / n) * (sW * sG).sum() + sW.sum() * sG.sum() / (n * n)
    loss = -T / ((n - 1) ** 2)
    return np.float32(loss)


# revision 34
# speedup vs baseline: 2.2810x; 1.0848x over previous
"""HSIC loss kernel for 8 TRN2 NeuronCores.

Math: loss = -tr(CKW.CKG)/(n-1)^2 with CKX = KX.H, H = I - 1/n.
Expanded:  T = S1 - (2/n) sum_i sW_i sG_i + SW SG/n^2, loss = -T/(n-1)^2
where S1 = sum_ij KW.KG, sX = row sums of KX (KX symmetric).

Symmetry: only the region R = {(i,j): j >= 512*floor(i/512)} of each 4096^2
kernel block is computed (144 [128,512] tiles per matrix instead of 256).
For elements below R, the mirror (strictly-upper 512-blocks) supplies them:
S1 doubles those tiles' contributions, and row sums get the mirrored part
from COLUMN sums of the computed tiles (ones-vector matmuls into PSUM).

Sharding: [128,512]-tile-rows r=0..31; core c owns rows {2c,2c+1,30-2c,31-2c}
= 18 (r,chunk) pairs/core, a perfectly balanced split. The SPMD program is
IDENTICAL on every core: 18 uniform steps; all per-core variation is data
(lhsT/rhs/aug streams staged in compute order by the host).

Per step (one (r,jc) pair, W and G halves side by side in PSUM [128,1024]):
fp8(e4m3) DoubleRow matmuls: 1 aug matmul (K=12: 4-way fp8 splits of
a_j = -sq_j/2 for both halves + per-row a_i via ones-selector rows) then
2 DR matmuls per half (K=256 each) accumulate the dot products. ACT does one
pair-wide Exp -> tau (f16). DVE: custom POWSUM4 (s = t^2+t^4+t^8+t^16), then
per half a 4x-mode scalar_tensor_tensor k = tau + s with accum = rowsum(k),
then one 4x STT kW*kG with accum -> S1 partials. A ones[128,1] matmul per
group accumulates column sums into per-group PSUM rows (partition 32-aligned
slots, flushed by 3 ACT copies). Host combines everything in f64 and replaces
the (quantized) diagonal with its exact value.
"""
import numpy as np
import ml_dtypes
from contextlib import ExitStack

import concourse.bass as bass
import concourse.tile as tile
from concourse import bacc, mybir
import concourse.dve_ops as dve_ops
from concourse.dve_spec import Spec, Src0, lower, _has_src1
from concourse.dve_ops import DveOp
from concourse.dve_uop import DveOpSpec

N_ROWS = 4096
TUNE_LAG = 3
TUNE_KPP = 5
ABLATE = set()
FLUSH_POOL = False
D = 512
NCORES = 8
P = 128
NSTEP = 18
NG = 9
F8 = ml_dtypes.float8_e4m3
LAST_RESULT = None
LAST_SCALE = None

f32 = mybir.dt.float32
f16 = mybir.dt.float16
f8e4 = mybir.dt.float8e4
DR = mybir.MatmulPerfMode.DoubleRow
ADD = mybir.AluOpType.add
MULT = mybir.AluOpType.mult


def _ref_powsum5(in0, in1, s0, s1, imm2):
    t = in0.astype(np.float32)
    t2 = t * t
    t4 = t2 * t2
    t8 = t4 * t4
    return (((t + t2) + (t4 + t8)) + t8 * t8).astype(np.float32)


def _register_powsum5():
    name = "POWSUM5_HSIC_ANT"
    for op in dve_ops.OPS:
        if op.name == name:
            return op
    t = Src0
    t2 = t * t
    t4 = t2 * t2
    t8 = t4 * t4
    spec = Spec(body=((t + t2) + (t4 + t8)) + t8 * t8, reference=_ref_powsum5)
    shas = {}
    for ver in ("v3", "v4"):
        tmp = DveOpSpec(name=name, opcode=1, uops=lower(spec, ver=ver),
                        rd1_en=_has_src1(spec))
        shas[ver] = tmp.sha(ver)
    op = DveOp(name, spec, subdim=False, uops_sha=shas)
    dve_ops.OPS.append(op)
    dve_ops._SUB_OPCODE_FOR_NAME[name] = (
        dve_ops._CUSTOM_DVE_ROW_BASE + len(dve_ops.OPS) - 1)
    dve_ops.CUSTOM_DVE_SPECS[name] = op.spec
    return op


def _schedule(c):
    """18 (tile_row, chunk, strict_upper) steps for core c. The 4 straddle
    pairs (jc == block row: counted once, no colsum mirror) come FIRST as
    groups 0-1, then the 14 strict-upper pairs chunk-major as groups 2-8.
    This fixed straddle/upper step layout is identical on every core, so the
    two S1 PSUM accumulators can be routed by step index in the uniform
    SPMD program. Groups (consecutive step pairs) always share the chunk."""
    rows = [2 * c, 2 * c + 1, 30 - 2 * c, 31 - 2 * c]
    straddle = []
    upper = []
    for jc in range(8):
        for r in rows:
            if jc == r // 4:
                straddle.append((r, jc, False))
            elif jc > r // 4:
                upper.append((r, jc, True))
    steps = straddle + upper
    assert len(straddle) == 4 and len(steps) == NSTEP
    for g in range(NG):
        assert steps[2 * g][1] == steps[2 * g + 1][1]
        assert steps[2 * g][2] == steps[2 * g + 1][2]
    return steps


def _build(scale: float):
    POWSUM5 = _register_powsum5()
    nc = bacc.Bacc("TRN2", target_bir_lowering=False, debug=False)

    rwg_d = nc.dram_tensor("rwg", [P, NG * 4096], f8e4, kind="ExternalInput")
    lwg_d = nc.dram_tensor("lwg", [P, NG * 2048], f8e4, kind="ExternalInput")
    arl_d = nc.dram_tensor("arl", [12, NG * 2560], f8e4, kind="ExternalInput")
    id_d = nc.dram_tensor("ident", [P, 128], f16, kind="ExternalInput")
    acc_d = nc.dram_tensor("acc", [P, 2 * NSTEP + 2], f32, kind="ExternalOutput")
    cs_d = nc.dram_tensor("cs", [12, 1024], f32, kind="ExternalOutput")

    with tile.TileContext(nc) as tc, ExitStack() as ctx:
        const = ctx.enter_context(tc.tile_pool(name="const", bufs=1))
        psum = ctx.enter_context(tc.tile_pool(name="psum", bufs=2, space="PSUM"))
        csp = ctx.enter_context(tc.tile_pool(name="csp", bufs=1, space="PSUM"))
        taup = ctx.enter_context(tc.tile_pool(name="taup", bufs=3))
        kpp = ctx.enter_context(tc.tile_pool(name="kpp", bufs=TUNE_KPP))
        dmp = ctx.enter_context(tc.tile_pool(name="dmp", bufs=4))

        rwg_t = const.tile([P, NG * 4096], f8e4, tag="rwg", name="rwg_t")
        lwg_t = const.tile([P, NG * 2048], f8e4, tag="lwg", name="lwg_t")
        arl_t = const.tile([12, NG * 2560], f8e4, tag="arl", name="arl_t")
        ones_t = const.tile([P, 1], f16, tag="ones", name="ones_t")
        acc_t = const.tile([P, 2 * NSTEP + 2], f32, tag="acc", name="acc_t")
        ident_t = const.tile([P, 128], f16, tag="ident", name="ident_t")
        stage = [const.tile([65, 1024], f32, tag=f"st{i}", name=f"st{i}")
                 for i in range(0 if "cs" in ABLATE else 3)]
        nc.vector.memset(ones_t[:], 1.0)
        # per-group prefetch in compute order: 3 combined DMAs per group
        for g in range(NG):
            if g == 2:
                nc.sync.dma_start(ident_t[:], id_d.ap()[:])
            nc.sync.dma_start(arl_t[:, g * 2560:(g + 1) * 2560],
                              arl_d.ap()[:, g * 2560:(g + 1) * 2560])
            nc.sync.dma_start(lwg_t[:, g * 2048:(g + 1) * 2048],
                              lwg_d.ap()[:, g * 2048:(g + 1) * 2048])
            nc.sync.dma_start(rwg_t[:, g * 4096:(g + 1) * 4096],
                              rwg_d.ap()[:, g * 4096:(g + 1) * 4096])

        cs_tile = None if "cs" in ABLATE else csp.tile([65, 1024], f32, tag="cs0", name="cs0")
        s1_str = None if "s1" in ABLATE else csp.tile([P, 512], f32, tag="s1a", name="s1a")
        s1_upp = None if "s1" in ABLATE else csp.tile([P, 512], f32, tag="s1b", name="s1b")
        LAG = TUNE_LAG
        kp_list = {}
        flush_state = [0]
        pending_flush = []

        def emit_step(s):
            g, u = s // 2, s % 2
            ps = psum.tile([P, 1024], f32, tag="pair", name="pair")
            al_ap = arl_t[:, g * 2560 + 2048 + u * 256:
                          g * 2560 + 2048 + (u + 1) * 256].rearrange(
                "p (two m) -> p two m", two=2)
            for h in range(2):
                ar_ap = arl_t[:, g * 2560 + h * 1024:g * 2560 + (h + 1) * 1024] \
                    .rearrange("p (two n) -> p two n", two=2)
                nc.tensor.matmul(ps[:, h * 512:(h + 1) * 512], al_ap, ar_ap,
                                 start=True, stop=False, perf_mode=DR)
            for h in range(2):
                lbase = g * 2048 + h * 1024 + u * 512
                rbase = g * 4096 + h * 2048
                for kc in range(2):
                    lap = lwg_t[:, lbase + kc * 256:lbase + (kc + 1) * 256] \
                        .rearrange("p (two m) -> p two m", two=2)
                    rap = rwg_t[:, rbase + kc * 1024:rbase + (kc + 1) * 1024] \
                        .rearrange("p (two n) -> p two n", two=2)
                    nc.tensor.matmul(ps[:, h * 512:(h + 1) * 512], lap, rap,
                                     start=False, stop=(kc == 1), perf_mode=DR)
            tau = taup.tile([P, 1024], f16, tag="tau", name="tau")
            nc.scalar.activation(tau[:], ps[:],
                                 mybir.ActivationFunctionType.Exp,
                                 bias=0.0, scale=scale)
            while pending_flush:
                i, tile_ref = pending_flush.pop(0)
                if FLUSH_POOL:
                    nc.gpsimd.tensor_copy(stage[i][:], tile_ref[:])
                else:
                    nc.scalar.copy(stage[i][:], tile_ref[:])
                nc.sync.dma_start(cs_d.ap()[3 * i:3 * i + 3, :],
                                  stage[i][0:65:32, :])
            kp = kpp.tile([P, 1024], f16, tag="kp", name="kp")
            if "pow" not in ABLATE:
                nc.vector._custom_dve(POWSUM5, out=kp[:], in0=tau[:])
            else:
                nc.vector.tensor_copy(kp[:], tau[:])
            for h in range(2):
                if "red" in ABLATE:
                    break
                sl = slice(h * 512, (h + 1) * 512)
                dummy = dmp.tile([P, 512], f16, tag="dm", name="dm")
                nc.vector.tensor_scalar(
                    out=dummy[:], in0=kp[:, sl], scalar1=1.0, scalar2=0.0,
                    op0=MULT, op1=ADD,
                    accum_out=acc_t[:, 2 * s + h:2 * s + h + 1])
            kp_list[s] = kp

        def emit_lagged(s):
            g, u = s // 2, s % 2
            kp = kp_list.pop(s)
            # S1: accumulate kW_q^T . kG_q; its diagonal sums to sum(kW*kG).
            # Straddle steps (0-3) and strict-upper steps (4-17) use separate
            # accumulators (host weights them 1x / 2x).
            accT = s1_str if s < 4 else s1_upp
            first = (s == 0) if s < 4 else (s == 4)
            last = (s == 3) if s < 4 else (s == NSTEP - 1)
            for q in range(4 if "s1" not in ABLATE else 0):
                nc.tensor.matmul(accT[:, 0:128],
                                 kp[:, q * 128:(q + 1) * 128],
                                 kp[:, 512 + q * 128:512 + (q + 1) * 128],
                                 start=(first and q == 0), stop=(last and q == 3),
                                 skip_group_check=True)
            # column sums into per-group PSUM row (32-aligned slot)
            if s >= 4 and "cs" not in ABLATE:
                qrow = (g % 3) * 32
                for h in range(2):
                    nc.tensor.matmul(
                        cs_tile[qrow:qrow + 1, h * 512:(h + 1) * 512],
                        ones_t[:], kp[:, h * 512:(h + 1) * 512],
                        start=(u == 0), stop=(u == 1), skip_group_check=True)
            if u == 1 and g in (2, 5, 8) and "cs" not in ABLATE:
                pending_flush.append((flush_state[0], cs_tile))
                flush_state[0] += 1

        for s in range(NSTEP):
            if s >= LAG:
                emit_lagged(s - LAG)
            emit_step(s)
        for s in range(NSTEP - LAG, NSTEP):
            emit_lagged(s)
        while pending_flush:
            i, tile_ref = pending_flush.pop(0)
            nc.scalar.copy(stage[i][:], tile_ref[:])
            nc.sync.dma_start(cs_d.ap()[3 * i:3 * i + 3, :],
                              stage[i][0:65:32, :])
        nc.sync.dma_start(acc_d.ap()[:, 0:2 * NSTEP - 4],
                          acc_t[:, 0:2 * NSTEP - 4])
        for i, accT in enumerate(() if "s1" in ABLATE else (s1_str, s1_upp)):
            ddump = dmp.tile([P, 128], f32, tag="dd", name="dd")
            nc.vector.scalar_tensor_tensor(
                out=ddump[:], in0=accT[:, 0:128], scalar=1.0, in1=ident_t[:],
                op0=MULT, op1=MULT,
                accum_out=acc_t[:, 2 * NSTEP + i:2 * NSTEP + i + 1])
        nc.sync.dma_start(acc_d.ap()[:, 2 * NSTEP - 4:],
                          acc_t[:, 2 * NSTEP - 4:])
    nc.compile()
    return nc


def _split4(x):
    """4-term fp8 split of x (f64): sum of returned rows ~ x."""
    outs = []
    r = x.copy()
    for _ in range(4):
        h = r.astype(F8)
        outs.append(h)
        r = r - h.astype(np.float64)
    return outs


def _k16_of_tau(tau16):
    """Device-replica: k16 = f16(powsum5_f32(f16 tau)); every consumer
    (rowsum reduce, S1 product, colsum matmul) reads this same value."""
    t = tau16.astype(np.float32)
    t2 = t * t
    t4 = t2 * t2
    t8 = t4 * t4
    k = (((t + t2) + (t4 + t8)) + t8 * t8).astype(np.float16)
    return k.astype(np.float64)


def kernel(W, G, **_):
    import os
    os.environ["BASS_NEVER_TRACE"] = "1"
    from concourse.bass_utils import run_bass_kernel_spmd
    W = np.asarray(W, dtype=np.float32)
    G = np.asarray(G, dtype=np.float32)
    n = W.shape[0]
    N = 2 * n

    # bandwidth from the full-precision inputs (closed form, f64)
    W64, G64 = W.astype(np.float64), G.astype(np.float64)
    sqW_t = (W64 * W64).sum(1)
    sqG_t = (G64 * G64).sum(1)
    colsum = W64.sum(0) + G64.sum(0)
    sum_d2 = 2.0 * N * (sqW_t.sum() + sqG_t.sum()) - 2.0 * (colsum * colsum).sum()
    bw = sum_d2 / (N * N - N) / 4.0
    scale = float(np.float32(1.0 / (8.0 * bw)))

    # fp8 quantization + aug splits (from quantized rows: keeps d2_q >= 0
    # and the diagonal exactly zero pre-rounding)
    W8 = W.astype(F8)
    G8 = G.astype(F8)
    W8f = W8.astype(np.float64)
    G8f = G8.astype(np.float64)
    aW = -0.5 * (W8f * W8f).sum(1)
    aG = -0.5 * (G8f * G8f).sum(1)
    # 4-term fp8 split of a/2 (e4m3 max is 240; |a| can exceed it), applied
    # through selector rows of 2.0 in the aug matmul.
    aW4 = _split4(aW / 2.0)
    aG4 = _split4(aG / 2.0)
    aWs = 2.0 * sum(a.astype(np.float64) for a in aW4)
    aGs = 2.0 * sum(a.astype(np.float64) for a in aG4)
    W8T = np.ascontiguousarray(W8.T)  # [feat, row]
    G8T = np.ascontiguousarray(G8.T)

    scheds = [_schedule(c) for c in range(NCORES)]
    in_maps = []
    for c in range(NCORES):
        st = scheds[c]
        rwg = np.zeros((P, NG * 4096), F8)
        lwg = np.zeros((P, NG * 2048), F8)
        arl = np.zeros((12, NG * 2560), F8)
        for g in range(NG):
            jc = st[2 * g][1]
            cols = slice(jc * 512, (jc + 1) * 512)
            for q in range(4):
                rwg[:, g * 4096 + q * 512:g * 4096 + (q + 1) * 512] = \
                    W8T[q * P:(q + 1) * P, cols]
                rwg[:, g * 4096 + 2048 + q * 512:g * 4096 + 2048 + (q + 1) * 512] = \
                    G8T[q * P:(q + 1) * P, cols]
                arl[q, g * 2560 + 0:g * 2560 + 512] = aW4[q][cols]
                arl[4 + q, g * 2560 + 0:g * 2560 + 512] = 2.0
                arl[q, g * 2560 + 1024:g * 2560 + 1536] = aG4[q][cols]
                arl[8 + q, g * 2560 + 1024:g * 2560 + 1536] = 2.0
        for s, (r, jc, su) in enumerate(st):
            g, u = s // 2, s % 2
            rsl = slice(r * P, (r + 1) * P)
            for kc in range(2):
                for i in range(2):
                    fsl = slice(kc * 256 + i * P, kc * 256 + (i + 1) * P)
                    base = g * 2048 + u * 512 + kc * 256 + i * P
                    lwg[:, base:base + P] = W8T[fsl, rsl]
                    lwg[:, 1024 + base:1024 + base + P] = G8T[fsl, rsl]
            abase = g * 2560 + 2048 + u * 256
            for q in range(4):
                arl[q, abase:abase + P] = 2.0
                arl[4 + q, abase:abase + P] = aW4[q][rsl]
                arl[8 + q, abase:abase + P] = aG4[q][rsl]
        in_maps.append({"rwg": rwg, "lwg": lwg, "arl": arl,
                        "ident": np.eye(P, dtype=np.float16)})

    global LAST_SCALE
    LAST_SCALE = scale
    nc = _build(scale)
    res = run_bass_kernel_spmd(nc, in_maps, core_ids=list(range(NCORES)))
    global LAST_RESULT
    LAST_RESULT = res

    # host combine (f64)
    S1 = 0.0
    sW = np.zeros(n)
    sG = np.zeros(n)
    for c in range(NCORES):
        out = res.results[c]
        acc = out["acc"].astype(np.float64)
        cs = out["cs"].astype(np.float64)
        for s, (r, jc, su) in enumerate(scheds[c]):
            rsl = slice(r * P, (r + 1) * P)
            sW[rsl] += acc[:, 2 * s + 0]
            sG[rsl] += acc[:, 2 * s + 1]
        S1 += acc[:, 2 * NSTEP].sum() + 2.0 * acc[:, 2 * NSTEP + 1].sum()
        for g in range(NG):
            jc, su = scheds[c][2 * g][1], scheds[c][2 * g][2]
            if su:
                csl = slice(jc * 512, (jc + 1) * 512)
                sW[csl] += cs[g, 0:512]
                sG[csl] += cs[g, 512:1024]

    # diagonal: replace device-computed quantized values with exact 5.0
    sc32 = np.float32(scale)
    for X8f, aXs, sX, which in ((W8f, aWs, sW, 0), (G8f, aGs, sG, 1)):
        g_ii = (X8f * X8f).sum(1)
        P_ii = (g_ii + 2.0 * aXs).astype(np.float32)
        tau = (np.exp(P_ii * sc32)).astype(np.float16)
        k16 = _k16_of_tau(tau)
        sX += 5.0 - k16
        if which == 0:
            kWd = k16
        else:
            kGd = k16
    S1 += (25.0 - kWd * kGd).sum()

    T = S1 - (2.0 / n) * (sW * sG).sum() + sW.sum() * sG.sum() / (n * n)
    loss = -T / ((n - 1) ** 2)
    return np.float32(loss)
